# revision 20
# baseline (speedup 1.0000x reference)
"""Trainium2 Bass kernel for a padded-attention transformer encoder layer.

Shapes (hardcoded): src [4, 2048, 1024], 16 heads x 64, d_ff 4096, 8 cores.

Sharding: each core computes the full layer for 1024 output tokens
(batch = core//2, token half = core%2). Inputs are host-transposed
(feature-major) and host-rolled so every core's query tokens are columns
0:1024 of its srcT; attention over keys is permutation invariant so the
roll only permutes the contraction order.

On-core dataflow (everything feature-on-partitions, fp32r matmuls):
  xT = rmsnorm_T(srcT)                    (partition-dim reduce via ones-matmul)
  qT2/kT2 per head pair from xT; v (token-major) via xT-stationary matmuls
  scoresT[t,s] = kT.T @ qT; expT = exp(0.125*scoresT)  (no max-sub needed)
  v_aug = [v*kmask | kmask]  -> attnV matmul yields o and softmax denom at once
  o normalized by qmask/denom, out-proj accumulated into saT += Wo.T-part
  yT = rmsnorm_T(saT); h1 = silu(yT@W1)*(yT@V1) stored bf16 over dead xT space
  outT = saT + h1 @ W2  (bf16 matmul, fp32 accum)
"""

import sys

sys.path.insert(0, "/opt/trn_rl_repo")

import numpy as np
import ml_dtypes

import concourse.bass as bass
import concourse.mybir as mybir
import concourse.tile as tile
from concourse import bacc
from concourse.bass_utils import run_bass_kernel_spmd

F32 = mybir.dt.float32
F32R = mybir.dt.float32r
BF16 = mybir.dt.bfloat16
EXP = mybir.ActivationFunctionType.Exp
SILU = mybir.ActivationFunctionType.Silu
SQRT = mybir.ActivationFunctionType.Sqrt

B, S, D, H, DK, DFF = 4, 2048, 1024, 16, 64, 4096
SQ = 1024          # query tokens per core
DC = D // 128      # 8 d-chunks
TC = S // 128      # 16 token chunks
FC = DFF // 128    # 32 dff chunks
NPAIR = H // 2     # 8 head pairs
SCALE = DK ** -0.5


def r32(ap):
    return ap.bitcast(F32R)


def build():
    nc = bacc.Bacc("TRN2", target_bir_lowering=False, debug=False, num_devices=8)

    srcT = nc.dram_tensor("srcT", [D, S], F32, kind="ExternalInput").ap()
    kmask = nc.dram_tensor("kmask", [S, 1], F32, kind="ExternalInput").ap()
    qmaskd = nc.dram_tensor("qmask", [1, SQ], F32, kind="ExternalInput").ap()
    wq = nc.dram_tensor("wq", [D, D], F32, kind="ExternalInput").ap()
    wk = nc.dram_tensor("wk", [D, D], F32, kind="ExternalInput").ap()
    wv = nc.dram_tensor("wv", [D, D], F32, kind="ExternalInput").ap()
    wo = nc.dram_tensor("wo", [D, D], F32, kind="ExternalInput").ap()
    w1 = nc.dram_tensor("w1", [D, DFF], F32, kind="ExternalInput").ap()
    v1 = nc.dram_tensor("v1", [D, DFF], F32, kind="ExternalInput").ap()
    w2b = nc.dram_tensor("w2b", [DFF, D], BF16, kind="ExternalInput").ap()
    outT = nc.dram_tensor("outT", [D, SQ], F32, kind="ExternalOutput").ap()

    # persistent SBUF arrays. h1 (bf16, FFN intermediate) aliases xt's bytes:
    # xt is fully consumed before the first h1 write (enforced by the tracked
    # byte-range deps through the saT chain), and the verifier needs separate
    # memory locations for the fp32r- and bf16-consumed data.
    xt, h1t = [], []
    for i in range(DC):
        xt.append(nc.alloc_sbuf_tensor(f"xt{i}", [128, S], F32R).ap())
        off = nc.sbuf_base - S * 4
        h1t.append(nc.alloc_sbuf_tensor_at(f"h1t{i}", [128, 2 * S], BF16, offset=off).ap())
    # sat holds the attention residual stream saT, then is scaled in place to
    # yT = rmsnorm_T(saT); the final residual is reconstructed as yT * rms.
    sat = [nc.alloc_sbuf_tensor(f"sat{i}", [128, SQ], F32R).ap() for i in range(DC)]
    # v for one quarter-round (2 pairs = 4 heads), augmented with kmask col
    vq = [nc.alloc_sbuf_tensor(f"vq{i}", [128, 4 * 65], F32R).ap() for i in range(TC)]

    with nc.allow_low_precision(reason="fp32r matmul operand rounding; fp32 PSUM accumulation"), \
         tile.TileContext(nc) as tc:
        with (
            tc.tile_pool(name="kt2p", bufs=2) as kt2p,
            tc.tile_pool(name="qt2p", bufs=2) as qt2p,
            tc.tile_pool(name="expp", bufs=3) as expp,
            tc.tile_pool(name="tmp", bufs=3) as tmp,
            tc.tile_pool(name="wst", bufs=8) as wst,
            tc.tile_pool(name="w2st", bufs=8) as w2st,
            tc.tile_pool(name="wost", bufs=4) as wost,
            tc.tile_pool(name="consts", bufs=1) as consts,
            tc.tile_pool(name="sm", bufs=1) as sm,
            tc.tile_pool(name="psA", bufs=2, space="PSUM") as psA,
            tc.tile_pool(name="psB", bufs=2, space="PSUM") as psB,
        ):
            # ---- constants ----
            onesf = consts.tile([128, 128], F32, tag="onesf")
            nc.vector.memset(onesf[:], 1.0)
            ones4 = consts.tile([128, 4], F32R, tag="ones4")
            nc.vector.tensor_copy(ones4[:], onesf[:, 0:4])
            ones1 = consts.tile([1, 128], F32R, tag="ones1")
            nc.vector.tensor_copy(ones1[:], onesf[0:1, :])
            km = consts.tile([128, TC], F32, tag="km")
            for ti in range(TC):
                nc.sync.dma_start(out=km[:, ti : ti + 1], in_=kmask[ti * 128 : (ti + 1) * 128, :])
            qm = consts.tile([1, SQ], F32, tag="qm")
            nc.sync.dma_start(out=qm[:], in_=qmaskd[:])

            # ---- P0: xT = rmsnorm_T(srcT), in two 1024-col halves ----
            for th in range(2):
                hs = slice(th * 1024, (th + 1) * 1024)
                ss = psB.tile([4, 1024], F32, tag="acc")
                for dc in range(DC):
                    ld = tmp.tile([128, 1024], F32R, tag="tmp")
                    nc.sync.dma_start(out=ld[:], in_=r32(srcT[dc * 128 : (dc + 1) * 128, hs]))
                    sq = tmp.tile([128, 1024], F32R, tag="tmp")
                    nc.vector.tensor_mul(sq[:], ld[:], ld[:])
                    for n2 in range(2):
                        ns = slice(n2 * 512, (n2 + 1) * 512)
                        nc.tensor.matmul(ss[:, ns], ones4[:], sq[:, ns],
                                         start=(dc == 0), stop=(dc == DC - 1))
                rms = sm.tile([1, 1024], F32R, tag="rms")
                nc.scalar.activation(rms[:], ss[0:1, :], SQRT, scale=1.0 / D)
                rmsB = psA.tile([128, 1024], F32, tag="mm")
                for n2 in range(2):
                    ns = slice(n2 * 512, (n2 + 1) * 512)
                    nc.tensor.matmul(rmsB[:, ns], ones1[:], rms[:, ns], start=True, stop=True)
                invB = tmp.tile([128, 1024], F32R, tag="tmp")
                nc.vector.reciprocal(invB[:], rmsB[:])
                for dc in range(DC):
                    ld2 = tmp.tile([128, 1024], F32R, tag="tmp")
                    nc.sync.dma_start(out=ld2[:], in_=r32(srcT[dc * 128 : (dc + 1) * 128, hs]))
                    nc.vector.tensor_mul(xt[dc][:, hs], ld2[:], invB[:])

            # ---- P1: attention ----
            for p in range(NPAIR):
                vr, lp = p // 2, p % 2
                if lp == 0:
                    # v for pairs {2vr, 2vr+1}: heads 4vr..4vr+3 (cols 256 of wv)
                    wvts = []
                    for dc in range(DC):
                        wvt = wst.tile([128, 256], F32R, tag="wst")
                        nc.sync.dma_start(
                            out=wvt[:],
                            in_=r32(wv[dc * 128 : (dc + 1) * 128, vr * 256 : (vr + 1) * 256]),
                        )
                        wvts.append(wvt)
                    for ti in range(TC):
                        vps = psA.tile([128, 1024], F32, tag="mm")
                        for dc in range(DC):
                            nc.tensor.matmul(
                                vps[:, 0:256],
                                xt[dc][:, ti * 128 : (ti + 1) * 128],
                                wvts[dc][:],
                                start=(dc == 0), stop=(dc == DC - 1),
                            )
                        # vq[ti] = [v*km | km] interleaved per head: [128, 4, 65]
                        dst = vq[ti].rearrange("p (h c) -> p h c", c=65)
                        src3 = vps[:, 0:256].rearrange("p (h c) -> p h c", c=64)
                        nc.vector.tensor_scalar_mul(dst[:, :, 0:64], src3, km[:, ti : ti + 1])
                        for l in range(4):
                            nc.vector.tensor_copy(dst[:, l, 64:65], r32(km[:, ti : ti + 1]))

                # kT2 / qT2 for this pair
                kps = [psA.tile([128, 1024], F32, tag="mm", name=f"kps{half}")
                       for half in range(2)]
                qps = psB.tile([128, 1024], F32, tag="acc")
                for dc in range(DC):
                    wkt = wst.tile([128, 128], F32R, tag="wst")
                    nc.sync.dma_start(out=wkt[:], in_=r32(wk[dc * 128 : (dc + 1) * 128, p * 128 : (p + 1) * 128]))
                    wqt = wst.tile([128, 128], F32R, tag="wst")
                    nc.sync.dma_start(out=wqt[:], in_=r32(wq[dc * 128 : (dc + 1) * 128, p * 128 : (p + 1) * 128]))
                    for half in range(2):
                        for n2 in range(2):
                            ns = slice(n2 * 512, (n2 + 1) * 512)
                            nc.tensor.matmul(
                                kps[half][:, ns], wkt[:],
                                xt[dc][:, half * 1024 + n2 * 512 : half * 1024 + (n2 + 1) * 512],
                                start=(dc == 0), stop=(dc == DC - 1),
                            )
                    for n2 in range(2):
                        ns = slice(n2 * 512, (n2 + 1) * 512)
                        nc.tensor.matmul(qps[:, ns], wqt[:], xt[dc][:, ns],
                                         start=(dc == 0), stop=(dc == DC - 1))
                kt2 = kt2p.tile([128, S], F32R, tag="kt2")
                for half in range(2):
                    nc.vector.tensor_copy(kt2[:, half * 1024 : (half + 1) * 1024], kps[half][:])
                qt2 = qt2p.tile([128, SQ], F32R, tag="qt2")
                nc.vector.tensor_copy(qt2[:], qps[:])

                otn = tmp.tile([128, 1024], F32R, tag="tmp")
                for h in range(2):
                    l = 2 * lp + h
                    oT = psB.tile([128, 1024], F32, tag="acc")
                    for ti in range(TC):
                        sc = psA.tile([128, 1024], F32, tag="mm")
                        for n2 in range(2):
                            ns = slice(n2 * 512, (n2 + 1) * 512)
                            nc.tensor.matmul(
                                sc[:, ns],
                                kt2[h * 64 : (h + 1) * 64, ti * 128 : (ti + 1) * 128],
                                qt2[h * 64 : (h + 1) * 64, ns],
                                start=True, stop=True,
                            )
                        ex = expp.tile([128, 1024], F32R, tag="exp")
                        nc.scalar.activation(ex[:], sc[:], EXP, scale=SCALE)
                        for n2 in range(2):
                            ns = slice(n2 * 512, (n2 + 1) * 512)
                            nc.tensor.matmul(
                                oT[0:65, ns],
                                vq[ti][:, l * 65 : (l + 1) * 65],
                                ex[:, ns],
                                start=(ti == 0), stop=(ti == TC - 1),
                            )
                    # normalize rows: o/denom, with denom+=1e30 on masked queries
                    rsq = sm.tile([1, 1024], F32R, tag="rsq")
                    nc.vector.tensor_add(rsq[:], oT[64:65, :], qm[:])
                    rB = psA.tile([128, 1024], F32, tag="mm")
                    for n2 in range(2):
                        ns = slice(n2 * 512, (n2 + 1) * 512)
                        nc.tensor.matmul(rB[0:64, ns], ones1[:, 0:64], rsq[:, ns],
                                         start=True, stop=True)
                    recB = tmp.tile([128, 1024], F32R, tag="tmp")
                    nc.vector.reciprocal(recB[0:64, :], rB[0:64, :])
                    nc.vector.tensor_mul(otn[h * 64 : (h + 1) * 64, :], oT[0:64, :], recB[0:64, :])

                # out-proj partial: saT[dc2] (+)= wo[p-rows, dc2-cols].T @ otn
                for dc2 in range(DC):
                    wot = wost.tile([128, 128], F32R, tag="wost")
                    nc.sync.dma_start(out=wot[:], in_=r32(wo[p * 128 : (p + 1) * 128, dc2 * 128 : (dc2 + 1) * 128]))
                    pp = psA.tile([128, 1024], F32, tag="mm")
                    for n2 in range(2):
                        ns = slice(n2 * 512, (n2 + 1) * 512)
                        nc.tensor.matmul(pp[:, ns], wot[:], otn[:, ns], start=True, stop=True)
                    if p == 0:
                        srcq = tmp.tile([128, 1024], F32, tag="tmp")
                        nc.sync.dma_start(out=srcq[:], in_=srcT[dc2 * 128 : (dc2 + 1) * 128, 0:SQ])
                        nc.vector.tensor_add(sat[dc2][:], pp[:], srcq[:])
                    else:
                        nc.vector.tensor_add(sat[dc2][:], sat[dc2][:], pp[:])

            # ---- P2: saT -> yT in place; keep rmsy for the final residual ----
            ssy = psB.tile([4, 1024], F32, tag="acc")
            for dc in range(DC):
                sqy = tmp.tile([128, 1024], F32R, tag="tmp")
                nc.vector.tensor_mul(sqy[:], sat[dc][:], sat[dc][:])
                for n2 in range(2):
                    ns = slice(n2 * 512, (n2 + 1) * 512)
                    nc.tensor.matmul(ssy[:, ns], ones4[:], sqy[:, ns],
                                     start=(dc == 0), stop=(dc == DC - 1))
            rmsy = consts.tile([1, 1024], F32R, tag="rmsy")
            nc.scalar.activation(rmsy[:], ssy[0:1, :], SQRT, scale=1.0 / D)
            rmsyB2 = psA.tile([128, 1024], F32, tag="mm")
            for n2 in range(2):
                ns = slice(n2 * 512, (n2 + 1) * 512)
                nc.tensor.matmul(rmsyB2[:, ns], ones1[:], rmsy[:, ns], start=True, stop=True)
            invyB = tmp.tile([128, 1024], F32R, tag="tmp")
            nc.vector.reciprocal(invyB[:], rmsyB2[:])
            for dc in range(DC):
                nc.vector.tensor_mul(sat[dc][:], sat[dc][:], invyB[:])

            # ---- P3: h1 = silu(yT@W1)*(yT@V1) -> bf16, overlaid on xt space ----
            h1c = []
            for fc in range(FC):
                h1c.append(h1t[fc // 4][:, (fc % 4) * 1024 : (fc % 4 + 1) * 1024])
            for fc in range(FC):
                h1w = psA.tile([128, 1024], F32, tag="mm")
                h1v = psB.tile([128, 1024], F32, tag="acc")
                for dc in range(DC):
                    w1t = wst.tile([128, 128], F32R, tag="wst")
                    nc.sync.dma_start(out=w1t[:], in_=r32(w1[dc * 128 : (dc + 1) * 128, fc * 128 : (fc + 1) * 128]))
                    v1t = wst.tile([128, 128], F32R, tag="wst")
                    nc.sync.dma_start(out=v1t[:], in_=r32(v1[dc * 128 : (dc + 1) * 128, fc * 128 : (fc + 1) * 128]))
                    for n2 in range(2):
                        ns = slice(n2 * 512, (n2 + 1) * 512)
                        nc.tensor.matmul(h1w[:, ns], w1t[:], sat[dc][:, ns],
                                         start=(dc == 0), stop=(dc == DC - 1))
                        nc.tensor.matmul(h1v[:, ns], v1t[:], sat[dc][:, ns],
                                         start=(dc == 0), stop=(dc == DC - 1))
                sil = tmp.tile([128, 1024], F32, tag="tmp")
                nc.scalar.activation(sil[:], h1w[:], SILU)
                nc.vector.tensor_mul(h1c[fc], sil[:], h1v[:])

            # ---- P4: outT = yT*rmsy + h1 @ W2   (saT reconstructed) ----
            rmsyB = psB.tile([128, 1024], F32, tag="acc")
            for n2 in range(2):
                ns = slice(n2 * 512, (n2 + 1) * 512)
                nc.tensor.matmul(rmsyB[:, ns], ones1[:], rmsy[:, ns], start=True, stop=True)
            for dc2 in range(DC):
                h2 = psA.tile([128, 1024], F32, tag="mm")
                for fc in range(FC):
                    w2t = w2st.tile([128, 128], BF16, tag="w2st")
                    nc.sync.dma_start(out=w2t[:], in_=w2b[fc * 128 : (fc + 1) * 128, dc2 * 128 : (dc2 + 1) * 128])
                    for n2 in range(2):
                        ns = slice(n2 * 512, (n2 + 1) * 512)
                        nc.tensor.matmul(h2[:, ns], w2t[:], h1c[fc][:, ns],
                                         start=(fc == 0), stop=(fc == FC - 1))
                sa_rec = tmp.tile([128, 1024], F32, tag="tmp")
                nc.vector.tensor_mul(sa_rec[:], sat[dc2][:], rmsyB[:])
                ot = tmp.tile([128, 1024], F32, tag="tmp")
                nc.vector.tensor_add(ot[:], sa_rec[:], h2[:])
                nc.sync.dma_start(out=outT[dc2 * 128 : (dc2 + 1) * 128, :], in_=ot[:])

    nc.compile()
    return nc


_NC = None


def _get_nc():
    global _NC
    if _NC is None:
        _NC = build()
    return _NC


def _build_in_maps(inputs):
    return _prep(**inputs)


def kernel(**inputs):
    in_maps = _prep(**inputs)
    res = run_bass_kernel_spmd(_get_nc(), in_maps, list(range(8)))
    out = np.empty((B, S, D), np.float32)
    for c in range(8):
        b, qh = c // 2, c % 2
        out[b, qh * SQ : (qh + 1) * SQ, :] = res.results[c]["outT"].T
    return out


def _prep(src, src_padding_mask, Wq, Wk, Wv, Wo, g1, g2, W1, V1, W2, **_):
    src = np.asarray(src, np.float32)
    valid = (~np.asarray(src_padding_mask, bool)).astype(np.float32)
    g1 = np.asarray(g1, np.float32)
    g2 = np.asarray(g2, np.float32)
    wq_cat = (np.transpose(np.asarray(Wq, np.float32), (1, 0, 2)).reshape(D, D)
              * g1[:, None]).astype(np.float32)
    wk_cat = (np.transpose(np.asarray(Wk, np.float32), (1, 0, 2)).reshape(D, D)
              * g1[:, None]).astype(np.float32)
    wv_cat = (np.transpose(np.asarray(Wv, np.float32), (1, 0, 2)).reshape(D, D)
              * g1[:, None]).astype(np.float32)
    wo_a = np.ascontiguousarray(np.asarray(Wo, np.float32))
    w1_s = np.ascontiguousarray(np.asarray(W1, np.float32) * g2[:, None])
    v1_s = np.ascontiguousarray(np.asarray(V1, np.float32) * g2[:, None])
    w2_b = np.asarray(W2, np.float32).astype(ml_dtypes.bfloat16)

    in_maps = []
    for c in range(8):
        b, qh = c // 2, c % 2
        roll = qh * SQ
        src_r = np.roll(src[b], -roll, axis=0)          # [S, D]
        srcT_c = np.ascontiguousarray(src_r.T)          # [D, S]
        km_c = np.ascontiguousarray(np.roll(valid[b], -roll)).reshape(S, 1)
        # additive softmax-denominator bias: +1e30 on padded query rows so
        # 1/denom ~ 0 there (reference zeroes those attention rows)
        qm_c = np.ascontiguousarray(((1.0 - km_c[0:SQ]) * 1e30).reshape(1, SQ))
        in_maps.append({
            "srcT": srcT_c, "kmask": km_c, "qmask": qm_c,
            "wq": wq_cat, "wk": wk_cat, "wv": wv_cat, "wo": wo_a,
            "w1": w1_s, "v1": v1_s, "w2b": w2_b,
        })
    return in_maps


# revision 27
# speedup vs baseline: 1.0829x; 1.0829x over previous
"""Trainium2 Bass kernel for a padded-attention transformer encoder layer.

Shapes (hardcoded): src [4, 2048, 1024], 16 heads x 64, d_ff 4096, 8 cores.

Sharding: each core computes the full layer for 1024 output tokens
(batch = core//2, token half = core%2). Inputs are host-transposed
(feature-major) and host-rolled so every core's query tokens are columns
0:1024 of its srcT; attention over keys is permutation invariant so the
roll only permutes the contraction order.

On-core dataflow (everything feature-on-partitions, fp32r matmuls):
  xT = rmsnorm_T(srcT)                    (partition-dim reduce via ones-matmul)
  qT2/kT2 per head pair from xT; v (token-major) via xT-stationary matmuls
  scoresT[t,s] = kT.T @ qT; expT = exp(0.125*scoresT)  (no max-sub needed)
  v_aug = [v*kmask | kmask]  -> attnV matmul yields o and softmax denom at once
  o normalized by qmask/denom, out-proj accumulated into saT += Wo.T-part
  yT = rmsnorm_T(saT); h1 = silu(yT@W1)*(yT@V1) stored bf16 over dead xT space
  outT = saT + h1 @ W2  (bf16 matmul, fp32 accum)
"""

import sys

sys.path.insert(0, "/opt/trn_rl_repo")

import numpy as np
import ml_dtypes

import concourse.bass as bass
import concourse.mybir as mybir
import concourse.tile as tile
from concourse import bacc
from concourse.bass_utils import run_bass_kernel_spmd

F32 = mybir.dt.float32
F32R = mybir.dt.float32r
BF16 = mybir.dt.bfloat16
EXP = mybir.ActivationFunctionType.Exp
SILU = mybir.ActivationFunctionType.Silu
SQRT = mybir.ActivationFunctionType.Sqrt

B, S, D, H, DK, DFF = 4, 2048, 1024, 16, 64, 4096
SQ = 1024          # query tokens per core
DC = D // 128      # 8 d-chunks
TC = S // 128      # 16 token chunks
FC = DFF // 128    # 32 dff chunks
NPAIR = H // 2     # 8 head pairs
SCALE = DK ** -0.5


def r32(ap):
    return ap.bitcast(F32R)


def build():
    nc = bacc.Bacc("TRN2", target_bir_lowering=False, debug=False, num_devices=8)

    srcT = nc.dram_tensor("srcT", [D, S], F32, kind="ExternalInput").ap()
    kmask = nc.dram_tensor("kmask", [S, 1], F32, kind="ExternalInput").ap()
    qmaskd = nc.dram_tensor("qmask", [1, SQ], F32, kind="ExternalInput").ap()
    wq = nc.dram_tensor("wq", [D, D], F32, kind="ExternalInput").ap()
    wk = nc.dram_tensor("wk", [D, D], F32, kind="ExternalInput").ap()
    wv = nc.dram_tensor("wv", [D, D], F32, kind="ExternalInput").ap()
    wo = nc.dram_tensor("wo", [D, D], F32, kind="ExternalInput").ap()
    w1 = nc.dram_tensor("w1", [D, DFF], F32, kind="ExternalInput").ap()
    v1 = nc.dram_tensor("v1", [D, DFF], F32, kind="ExternalInput").ap()
    w2b = nc.dram_tensor("w2b", [DFF, D], BF16, kind="ExternalInput").ap()
    outT = nc.dram_tensor("outT", [D, SQ], F32, kind="ExternalOutput").ap()

    # persistent SBUF arrays. h1 (bf16, FFN intermediate) aliases xt's bytes:
    # xt is fully consumed before the first h1 write (enforced by the tracked
    # byte-range deps through the saT chain), and the verifier needs separate
    # memory locations for the fp32r- and bf16-consumed data.
    xt, h1t = [], []
    for i in range(DC):
        xt.append(nc.alloc_sbuf_tensor(f"xt{i}", [128, S], F32R).ap())
        off = nc.sbuf_base - S * 4
        h1t.append(nc.alloc_sbuf_tensor_at(f"h1t{i}", [128, 2 * S], BF16, offset=off).ap())
    # sat holds the attention residual stream saT, then is scaled in place to
    # yT = rmsnorm_T(saT); the final residual is reconstructed as yT * rms.
    sat = [nc.alloc_sbuf_tensor(f"sat{i}", [128, SQ], F32R).ap() for i in range(DC)]
    # v for one quarter-round (2 pairs = 4 heads), augmented with kmask col
    vq = [nc.alloc_sbuf_tensor(f"vq{i}", [128, 4 * 65], F32R).ap() for i in range(TC)]

    with nc.allow_low_precision(reason="fp32r matmul operand rounding; fp32 PSUM accumulation"), \
         tile.TileContext(nc) as tc:
        with (
            tc.tile_pool(name="kt2p", bufs=2) as kt2p,
            tc.tile_pool(name="qt2p", bufs=2) as qt2p,
            tc.tile_pool(name="expp", bufs=3) as expp,
            tc.tile_pool(name="otnp", bufs=2) as otnp,
            tc.tile_pool(name="tmp", bufs=3) as tmp,
            tc.tile_pool(name="wst", bufs=8) as wst,
            tc.tile_pool(name="w2st", bufs=8) as w2st,
            tc.tile_pool(name="wost", bufs=4) as wost,
            tc.tile_pool(name="consts", bufs=1) as consts,
            tc.tile_pool(name="sm", bufs=1) as sm,
            tc.tile_pool(name="psA", bufs=2, space="PSUM") as psA,
            tc.tile_pool(name="psB", bufs=2, space="PSUM") as psB,
        ):
            # ---- constants ----
            onesf = consts.tile([128, 128], F32, tag="onesf")
            nc.vector.memset(onesf[:], 1.0)
            ones4 = consts.tile([128, 4], F32R, tag="ones4")
            nc.vector.tensor_copy(ones4[:], onesf[:, 0:4])
            ones1 = consts.tile([1, 128], F32R, tag="ones1")
            nc.vector.tensor_copy(ones1[:], onesf[0:1, :])
            km = consts.tile([128, TC], F32, tag="km")
            for ti in range(TC):
                nc.sync.dma_start(out=km[:, ti : ti + 1], in_=kmask[ti * 128 : (ti + 1) * 128, :])
            qm = consts.tile([1, SQ], F32, tag="qm")
            nc.sync.dma_start(out=qm[:], in_=qmaskd[:])

            # ---- P0: xT = rmsnorm_T(srcT), in two 1024-col halves ----
            for th in range(2):
                hs = slice(th * 1024, (th + 1) * 1024)
                ss = psB.tile([4, 1024], F32, tag="acc")
                for dc in range(DC):
                    ld = tmp.tile([128, 1024], F32R, tag="tmp")
                    nc.sync.dma_start(out=ld[:], in_=r32(srcT[dc * 128 : (dc + 1) * 128, hs]))
                    sq = tmp.tile([128, 1024], F32R, tag="tmp")
                    nc.vector.tensor_mul(sq[:], ld[:], ld[:])
                    for n2 in range(2):
                        ns = slice(n2 * 512, (n2 + 1) * 512)
                        nc.tensor.matmul(ss[:, ns], ones4[:], sq[:, ns],
                                         start=(dc == 0), stop=(dc == DC - 1))
                rms = sm.tile([1, 1024], F32R, tag="rms")
                nc.scalar.activation(rms[:], ss[0:1, :], SQRT, scale=1.0 / D)
                rmsB = psA.tile([128, 1024], F32, tag="mm")
                for n2 in range(2):
                    ns = slice(n2 * 512, (n2 + 1) * 512)
                    nc.tensor.matmul(rmsB[:, ns], ones1[:], rms[:, ns], start=True, stop=True)
                invB = tmp.tile([128, 1024], F32, tag="tmp")
                nc.vector.reciprocal_approx_fast(out=invB[:], in_=rmsB[:])
                for dc in range(DC):
                    ld2 = tmp.tile([128, 1024], F32R, tag="tmp")
                    nc.sync.dma_start(out=ld2[:], in_=r32(srcT[dc * 128 : (dc + 1) * 128, hs]))
                    nc.vector.tensor_mul(xt[dc][:, hs], ld2[:], invB[:])

            # ---- P1: attention ----
            # out-proj of pair p is emitted after pair p+1's kT2/qT2 matmuls so
            # the PE never stalls on the (DVE) softmax-denominator reciprocal.
            def emit_outproj(p, otn):
                for dc2 in range(DC):
                    wot = wost.tile([128, 128], F32R, tag="wost", name="wot")
                    nc.sync.dma_start(out=wot[:], in_=r32(wo[p * 128 : (p + 1) * 128, dc2 * 128 : (dc2 + 1) * 128]))
                    pp = psA.tile([128, 1024], F32, tag="mm", name="pp")
                    for n2 in range(2):
                        ns = slice(n2 * 512, (n2 + 1) * 512)
                        nc.tensor.matmul(pp[:, ns], wot[:], otn[:, ns], start=True, stop=True)
                    if p == 0:
                        srcq = tmp.tile([128, 1024], F32, tag="tmp", name="srcq")
                        nc.sync.dma_start(out=srcq[:], in_=srcT[dc2 * 128 : (dc2 + 1) * 128, 0:SQ])
                        nc.vector.tensor_add(sat[dc2][:], pp[:], srcq[:])
                    else:
                        nc.vector.tensor_add(sat[dc2][:], sat[dc2][:], pp[:])

            pending = None  # (pair index, otn tile) awaiting out-proj
            for p in range(NPAIR):
                vr, lp = p // 2, p % 2
                if lp == 0:
                    # v for pairs {2vr, 2vr+1}: heads 4vr..4vr+3 (cols 256 of wv)
                    wvts = []
                    for dc in range(DC):
                        wvt = wst.tile([128, 256], F32R, tag="wst")
                        nc.sync.dma_start(
                            out=wvt[:],
                            in_=r32(wv[dc * 128 : (dc + 1) * 128, vr * 256 : (vr + 1) * 256]),
                        )
                        wvts.append(wvt)
                    for ti in range(TC):
                        vps = psA.tile([128, 1024], F32, tag="mm")
                        for dc in range(DC):
                            nc.tensor.matmul(
                                vps[:, 0:256],
                                xt[dc][:, ti * 128 : (ti + 1) * 128],
                                wvts[dc][:],
                                start=(dc == 0), stop=(dc == DC - 1),
                            )
                        # vq[ti] = [v*km | km] interleaved per head: [128, 4, 65]
                        dst = vq[ti].rearrange("p (h c) -> p h c", c=65)
                        src3 = vps[:, 0:256].rearrange("p (h c) -> p h c", c=64)
                        nc.vector.tensor_scalar_mul(dst[:, :, 0:64], src3, km[:, ti : ti + 1])
                        for l in range(4):
                            nc.vector.tensor_copy(dst[:, l, 64:65], r32(km[:, ti : ti + 1]))

                # kT2 / qT2 for this pair
                kps = [psA.tile([128, 1024], F32, tag="mm", name=f"kps{half}")
                       for half in range(2)]
                qps = psB.tile([128, 1024], F32, tag="acc")
                for dc in range(DC):
                    wkt = wst.tile([128, 128], F32R, tag="wst")
                    nc.sync.dma_start(out=wkt[:], in_=r32(wk[dc * 128 : (dc + 1) * 128, p * 128 : (p + 1) * 128]))
                    wqt = wst.tile([128, 128], F32R, tag="wst")
                    nc.sync.dma_start(out=wqt[:], in_=r32(wq[dc * 128 : (dc + 1) * 128, p * 128 : (p + 1) * 128]))
                    for half in range(2):
                        for n2 in range(2):
                            ns = slice(n2 * 512, (n2 + 1) * 512)
                            nc.tensor.matmul(
                                kps[half][:, ns], wkt[:],
                                xt[dc][:, half * 1024 + n2 * 512 : half * 1024 + (n2 + 1) * 512],
                                start=(dc == 0), stop=(dc == DC - 1),
                            )
                    for n2 in range(2):
                        ns = slice(n2 * 512, (n2 + 1) * 512)
                        nc.tensor.matmul(qps[:, ns], wqt[:], xt[dc][:, ns],
                                         start=(dc == 0), stop=(dc == DC - 1))
                kt2 = kt2p.tile([128, S], F32R, tag="kt2")
                for half in range(2):
                    nc.vector.tensor_copy(kt2[:, half * 1024 : (half + 1) * 1024], kps[half][:])
                qt2 = qt2p.tile([128, SQ], F32R, tag="qt2")
                nc.vector.tensor_copy(qt2[:], qps[:])

                if pending is not None:
                    emit_outproj(*pending)
                otn = otnp.tile([128, 1024], F32R, tag="otn")
                for h in range(2):
                    l = 2 * lp + h
                    oT = psB.tile([128, 1024], F32, tag="acc")
                    for ti in range(TC):
                        sc = psA.tile([128, 1024], F32, tag="mm")
                        for n2 in range(2):
                            ns = slice(n2 * 512, (n2 + 1) * 512)
                            nc.tensor.matmul(
                                sc[:, ns],
                                kt2[h * 64 : (h + 1) * 64, ti * 128 : (ti + 1) * 128],
                                qt2[h * 64 : (h + 1) * 64, ns],
                                start=True, stop=True,
                            )
                        ex = expp.tile([128, 1024], F32R, tag="exp")
                        nc.scalar.activation(ex[:], sc[:], EXP, scale=SCALE)
                        for n2 in range(2):
                            ns = slice(n2 * 512, (n2 + 1) * 512)
                            nc.tensor.matmul(
                                oT[0:65, ns],
                                vq[ti][:, l * 65 : (l + 1) * 65],
                                ex[:, ns],
                                start=(ti == 0), stop=(ti == TC - 1),
                            )
                    # normalize rows: o/denom, with denom+=1e30 on masked queries
                    rsq = sm.tile([1, 1024], F32R, tag="rsq")
                    nc.vector.tensor_add(rsq[:], oT[64:65, :], qm[:])
                    rB = psA.tile([128, 1024], F32, tag="mm")
                    for n2 in range(2):
                        ns = slice(n2 * 512, (n2 + 1) * 512)
                        nc.tensor.matmul(rB[0:64, ns], ones1[:, 0:64], rsq[:, ns],
                                         start=True, stop=True)
                    recB = tmp.tile([128, 1024], F32, tag="tmp")
                    nc.vector.reciprocal_approx_fast(out=recB[0:64, :], in_=rB[0:64, :])
                    nc.vector.tensor_mul(otn[h * 64 : (h + 1) * 64, :], oT[0:64, :], recB[0:64, :])

                pending = (p, otn)
            emit_outproj(*pending)

            # ---- P2: saT -> yT in place; keep rmsy for the final residual ----
            ssy = psB.tile([4, 1024], F32, tag="acc")
            for dc in range(DC):
                sqy = tmp.tile([128, 1024], F32R, tag="tmp")
                nc.vector.tensor_mul(sqy[:], sat[dc][:], sat[dc][:])
                for n2 in range(2):
                    ns = slice(n2 * 512, (n2 + 1) * 512)
                    nc.tensor.matmul(ssy[:, ns], ones4[:], sqy[:, ns],
                                     start=(dc == 0), stop=(dc == DC - 1))
            rmsy = consts.tile([1, 1024], F32R, tag="rmsy")
            nc.scalar.activation(rmsy[:], ssy[0:1, :], SQRT, scale=1.0 / D)
            rmsyB2 = psA.tile([128, 1024], F32, tag="mm")
            for n2 in range(2):
                ns = slice(n2 * 512, (n2 + 1) * 512)
                nc.tensor.matmul(rmsyB2[:, ns], ones1[:], rmsy[:, ns], start=True, stop=True)
            invyB = tmp.tile([128, 1024], F32, tag="tmp")
            nc.vector.reciprocal_approx_fast(out=invyB[:], in_=rmsyB2[:])
            for dc in range(DC):
                nc.vector.tensor_mul(sat[dc][:], sat[dc][:], invyB[:])

            # ---- P3: h1 = silu(yT@W1)*(yT@V1) -> bf16, overlaid on xt space ----
            h1c = []
            for fc in range(FC):
                h1c.append(h1t[fc // 4][:, (fc % 4) * 1024 : (fc % 4 + 1) * 1024])
            for fc in range(FC):
                h1w = psA.tile([128, 1024], F32, tag="mm")
                h1v = psB.tile([128, 1024], F32, tag="acc")
                for dc in range(DC):
                    w1t = wst.tile([128, 128], F32R, tag="wst")
                    nc.sync.dma_start(out=w1t[:], in_=r32(w1[dc * 128 : (dc + 1) * 128, fc * 128 : (fc + 1) * 128]))
                    v1t = wst.tile([128, 128], F32R, tag="wst")
                    nc.sync.dma_start(out=v1t[:], in_=r32(v1[dc * 128 : (dc + 1) * 128, fc * 128 : (fc + 1) * 128]))
                    for n2 in range(2):
                        ns = slice(n2 * 512, (n2 + 1) * 512)
                        nc.tensor.matmul(h1w[:, ns], w1t[:], sat[dc][:, ns],
                                         start=(dc == 0), stop=(dc == DC - 1))
                        nc.tensor.matmul(h1v[:, ns], v1t[:], sat[dc][:, ns],
                                         start=(dc == 0), stop=(dc == DC - 1))
                sil = tmp.tile([128, 1024], F32, tag="tmp")
                nc.scalar.activation(sil[:], h1w[:], SILU)
                nc.vector.tensor_mul(h1c[fc], sil[:], h1v[:])

            # ---- P4: outT = yT*rmsy + h1 @ W2   (saT reconstructed) ----
            rmsyB = psB.tile([128, 1024], F32, tag="acc")
            for n2 in range(2):
                ns = slice(n2 * 512, (n2 + 1) * 512)
                nc.tensor.matmul(rmsyB[:, ns], ones1[:], rmsy[:, ns], start=True, stop=True)
            for dc2 in range(DC):
                h2 = psA.tile([128, 1024], F32, tag="mm")
                for fc in range(FC):
                    w2t = w2st.tile([128, 128], BF16, tag="w2st")
                    nc.sync.dma_start(out=w2t[:], in_=w2b[fc * 128 : (fc + 1) * 128, dc2 * 128 : (dc2 + 1) * 128])
                    for n2 in range(2):
                        ns = slice(n2 * 512, (n2 + 1) * 512)
                        nc.tensor.matmul(h2[:, ns], w2t[:], h1c[fc][:, ns],
                                         start=(fc == 0), stop=(fc == FC - 1))
                sa_rec = tmp.tile([128, 1024], F32, tag="tmp")
                nc.vector.tensor_mul(sa_rec[:], sat[dc2][:], rmsyB[:])
                ot = tmp.tile([128, 1024], F32, tag="tmp")
                nc.vector.tensor_add(ot[:], sa_rec[:], h2[:])
                nc.sync.dma_start(out=outT[dc2 * 128 : (dc2 + 1) * 128, :], in_=ot[:])

    nc.compile()
    return nc


_NC = None


def _get_nc():
    global _NC
    if _NC is None:
        _NC = build()
    return _NC


def _build_in_maps(inputs):
    return _prep(**inputs)


def kernel(**inputs):
    in_maps = _prep(**inputs)
    res = run_bass_kernel_spmd(_get_nc(), in_maps, list(range(8)))
    out = np.empty((B, S, D), np.float32)
    for c in range(8):
        b, qh = c // 2, c % 2
        out[b, qh * SQ : (qh + 1) * SQ, :] = res.results[c]["outT"].T
    return out


def _prep(src, src_padding_mask, Wq, Wk, Wv, Wo, g1, g2, W1, V1, W2, **_):
    src = np.asarray(src, np.float32)
    valid = (~np.asarray(src_padding_mask, bool)).astype(np.float32)
    g1 = np.asarray(g1, np.float32)
    g2 = np.asarray(g2, np.float32)
    wq_cat = (np.transpose(np.asarray(Wq, np.float32), (1, 0, 2)).reshape(D, D)
              * g1[:, None]).astype(np.float32)
    wk_cat = (np.transpose(np.asarray(Wk, np.float32), (1, 0, 2)).reshape(D, D)
              * g1[:, None]).astype(np.float32)
    wv_cat = (np.transpose(np.asarray(Wv, np.float32), (1, 0, 2)).reshape(D, D)
              * g1[:, None]).astype(np.float32)
    wo_a = np.ascontiguousarray(np.asarray(Wo, np.float32))
    w1_s = np.ascontiguousarray(np.asarray(W1, np.float32) * g2[:, None])
    v1_s = np.ascontiguousarray(np.asarray(V1, np.float32) * g2[:, None])
    w2_b = np.asarray(W2, np.float32).astype(ml_dtypes.bfloat16)

    in_maps = []
    for c in range(8):
        b, qh = c // 2, c % 2
        roll = qh * SQ
        src_r = np.roll(src[b], -roll, axis=0)          # [S, D]
        srcT_c = np.ascontiguousarray(src_r.T)          # [D, S]
        km_c = np.ascontiguousarray(np.roll(valid[b], -roll)).reshape(S, 1)
        # additive softmax-denominator bias: +1e30 on padded query rows so
        # 1/denom ~ 0 there (reference zeroes those attention rows)
        qm_c = np.ascontiguousarray(((1.0 - km_c[0:SQ]) * 1e30).reshape(1, SQ))
        in_maps.append({
            "srcT": srcT_c, "kmask": km_c, "qmask": qm_c,
            "wq": wq_cat, "wk": wk_cat, "wv": wv_cat, "wo": wo_a,
            "w1": w1_s, "v1": v1_s, "w2b": w2_b,
        })
    return in_maps


# revision 29
# speedup vs baseline: 1.1003x; 1.0161x over previous
"""Trainium2 Bass kernel for a padded-attention transformer encoder layer.

Shapes (hardcoded): src [4, 2048, 1024], 16 heads x 64, d_ff 4096, 8 cores.

Sharding: each core computes the full layer for 1024 output tokens
(batch = core//2, token half = core%2). Inputs are host-transposed
(feature-major) and host-rolled so every core's query tokens are columns
0:1024 of its srcT; attention over keys is permutation invariant so the
roll only permutes the contraction order.

On-core dataflow (feature-on-partitions, fp32r matmuls, PSUM in 1-bank
[128,512] slots for deep PE run-ahead):
  xT = rmsnorm_T(srcT)                    (partition-dim reduce via ones-matmul)
  qT2/kT2 per head pair from xT; v (token-major, bf16) via xT-stationary matmuls
  scoresT[t,s] = kT.T @ qT (fp32r); expT = exp(scores/8) in bf16
  v_aug = [v*kmask | kmask] -> attnV (bf16) yields o and softmax denom at once
  o normalized by 1/(denom + 1e30*qpad), out-proj accumulated into saT
  yT = rmsnorm_T(saT) in place; h1 = silu(yT@W1)*(yT@V1) bf16 over dead xT space
  outT = yT*rms + h1 @ W2  (bf16 matmul, fp32 accum)
"""

import sys

sys.path.insert(0, "/opt/trn_rl_repo")

import numpy as np
import ml_dtypes

import concourse.bass as bass
import concourse.mybir as mybir
import concourse.tile as tile
from concourse import bacc
from concourse.bass_utils import run_bass_kernel_spmd

F32 = mybir.dt.float32
F32R = mybir.dt.float32r
BF16 = mybir.dt.bfloat16
EXP = mybir.ActivationFunctionType.Exp
SILU = mybir.ActivationFunctionType.Silu
SQRT = mybir.ActivationFunctionType.Sqrt

B, S, D, H, DK, DFF = 4, 2048, 1024, 16, 64, 4096
SQ = 1024          # query tokens per core
DC = D // 128      # 8 d-chunks
TC = S // 128      # 16 token chunks
FC = DFF // 128    # 32 dff chunks
NPAIR = H // 2     # 8 head pairs
SCALE = DK ** -0.5
H0, H1 = slice(0, 512), slice(512, 1024)
HALVES = (H0, H1)


def r32(ap):
    return ap.bitcast(F32R)


def build():
    nc = bacc.Bacc("TRN2", target_bir_lowering=False, debug=False, num_devices=8)

    srcT = nc.dram_tensor("srcT", [D, S], F32, kind="ExternalInput").ap()
    kmask = nc.dram_tensor("kmask", [S, 1], F32, kind="ExternalInput").ap()
    qmaskd = nc.dram_tensor("qmask", [1, SQ], F32, kind="ExternalInput").ap()
    wq = nc.dram_tensor("wq", [D, D], F32, kind="ExternalInput").ap()
    wk = nc.dram_tensor("wk", [D, D], F32, kind="ExternalInput").ap()
    wv = nc.dram_tensor("wv", [D, D], F32, kind="ExternalInput").ap()
    wo = nc.dram_tensor("wo", [D, D], F32, kind="ExternalInput").ap()
    w1 = nc.dram_tensor("w1", [D, DFF], F32, kind="ExternalInput").ap()
    v1 = nc.dram_tensor("v1", [D, DFF], F32, kind="ExternalInput").ap()
    w2b = nc.dram_tensor("w2b", [DFF, D], BF16, kind="ExternalInput").ap()
    outT = nc.dram_tensor("outT", [D, SQ], F32, kind="ExternalOutput").ap()

    # persistent SBUF arrays. h1 (bf16, FFN intermediate) aliases xt's bytes:
    # xt is fully consumed before the first h1 write (guaranteed through the
    # tracked saT dependency chain) and the aliased tensors keep the fp32r-
    # and bf16-consumed memory locations distinct for the BIR verifier.
    xt, h1t = [], []
    for i in range(DC):
        xt.append(nc.alloc_sbuf_tensor(f"xt{i}", [128, S], F32R).ap())
        off = nc.sbuf_base - S * 4
        h1t.append(nc.alloc_sbuf_tensor_at(f"h1t{i}", [128, 2 * S], BF16, offset=off).ap())
    # saT: residual stream, scaled in place to yT; residual restored as yT*rms
    sat = [nc.alloc_sbuf_tensor(f"sat{i}", [128, SQ], F32R).ap() for i in range(DC)]
    # v for one half-round (4 pairs = 8 heads), bf16, kmask-augmented col 64
    vq = [nc.alloc_sbuf_tensor(f"vq{i}", [128, 8 * 65], BF16).ap() for i in range(TC)]

    with nc.allow_low_precision(reason="fp32r/bf16 matmul operand rounding; fp32 PSUM accumulation"), \
         tile.TileContext(nc) as tc:
        with (
            tc.tile_pool(name="kt2p", bufs=2) as kt2p,
            tc.tile_pool(name="qt2p", bufs=2) as qt2p,
            tc.tile_pool(name="expp", bufs=6) as expp,
            tc.tile_pool(name="otnp", bufs=2) as otnp,
            tc.tile_pool(name="tmp", bufs=3) as tmp,
            tc.tile_pool(name="wst", bufs=8) as wst,
            tc.tile_pool(name="w2st", bufs=8) as w2st,
            tc.tile_pool(name="wost", bufs=4) as wost,
            tc.tile_pool(name="consts", bufs=1) as consts,
            tc.tile_pool(name="sm", bufs=2) as sm,
            tc.tile_pool(name="ps", bufs=8, space="PSUM") as ps,
        ):
            # ---- constants ----
            onesf = consts.tile([128, 128], F32, tag="onesf")
            nc.vector.memset(onesf[:], 1.0)
            ones4 = consts.tile([128, 4], F32R, tag="ones4")
            nc.vector.tensor_copy(ones4[:], onesf[:, 0:4])
            ones1 = consts.tile([1, 128], F32R, tag="ones1")
            nc.vector.tensor_copy(ones1[:], onesf[0:1, :])
            km = consts.tile([128, TC], F32, tag="km")
            for ti in range(TC):
                nc.sync.dma_start(out=km[:, ti : ti + 1], in_=kmask[ti * 128 : (ti + 1) * 128, :])
            qm = consts.tile([1, SQ], F32, tag="qm")
            nc.sync.dma_start(out=qm[:], in_=qmaskd[:])

            # ---- P0: xT = rmsnorm_T(srcT), in two 1024-col halves ----
            for th in range(2):
                hs = slice(th * 1024, (th + 1) * 1024)
                ssq = [ps.tile([4, 512], F32, tag="ps", name=f"ssq{n2}") for n2 in range(2)]
                for dc in range(DC):
                    ld = tmp.tile([128, 1024], F32R, tag="tmp")
                    nc.sync.dma_start(out=ld[:], in_=r32(srcT[dc * 128 : (dc + 1) * 128, hs]))
                    sq = tmp.tile([128, 1024], F32R, tag="tmp")
                    nc.vector.tensor_mul(sq[:], ld[:], ld[:])
                    for n2 in range(2):
                        nc.tensor.matmul(ssq[n2][:], ones4[:], sq[:, HALVES[n2]],
                                         start=(dc == 0), stop=(dc == DC - 1))
                rms = consts.tile([1, 1024], F32R, tag=f"rms{th}")
                for n2 in range(2):
                    nc.scalar.activation(rms[:, HALVES[n2]], ssq[n2][0:1, :], SQRT, scale=1.0 / D)
                rmsB = [ps.tile([128, 512], F32, tag="ps", name=f"rmsB{n2}") for n2 in range(2)]
                for n2 in range(2):
                    nc.tensor.matmul(rmsB[n2][:], ones1[:], rms[:, HALVES[n2]], start=True, stop=True)
                invB = tmp.tile([128, 1024], F32, tag="tmp")
                for n2 in range(2):
                    nc.vector.reciprocal_approx_fast(out=invB[:, HALVES[n2]], in_=rmsB[n2][:])
                for dc in range(DC):
                    ld2 = tmp.tile([128, 1024], F32R, tag="tmp")
                    nc.sync.dma_start(out=ld2[:], in_=r32(srcT[dc * 128 : (dc + 1) * 128, hs]))
                    nc.vector.tensor_mul(xt[dc][:, hs], ld2[:], invB[:])

            # ---- P1: attention ----
            # out-proj of pair p is emitted after pair p+1's kT2/qT2 matmuls so
            # the PE never stalls on the softmax-denominator reciprocal chain.
            def emit_outproj(p, otn):
                for dc2 in range(DC):
                    wot = wost.tile([128, 128], F32R, tag="wost", name="wot")
                    nc.sync.dma_start(out=wot[:], in_=r32(wo[p * 128 : (p + 1) * 128, dc2 * 128 : (dc2 + 1) * 128]))
                    for n2 in range(2):
                        pp = ps.tile([128, 512], F32, tag="ps", name="pp")
                        nc.tensor.matmul(pp[:], wot[:], otn[:, HALVES[n2]], start=True, stop=True)
                        if p == 0:
                            srcq = tmp.tile([128, 512], F32, tag="tmp", name="srcq")
                            nc.sync.dma_start(out=srcq[:], in_=srcT[dc2 * 128 : (dc2 + 1) * 128, th_sq(n2)])
                            nc.vector.tensor_add(sat[dc2][:, HALVES[n2]], pp[:], srcq[:])
                        else:
                            nc.vector.tensor_add(sat[dc2][:, HALVES[n2]], sat[dc2][:, HALVES[n2]], pp[:])

            def th_sq(n2):
                return slice(n2 * 512, (n2 + 1) * 512)

            pending = None  # (pair index, otn tile) awaiting out-proj
            for p in range(NPAIR):
                vr, lp = p // 4, p % 4
                if lp == 0:
                    # v for pairs 4vr..4vr+3 (8 heads, 512 wv cols), bf16+kmask aug
                    wvts = []
                    for dc in range(DC):
                        wvt = wst.tile([128, 512], F32R, tag="wst", name="wvt")
                        nc.sync.dma_start(
                            out=wvt[:],
                            in_=r32(wv[dc * 128 : (dc + 1) * 128, vr * 512 : (vr + 1) * 512]),
                        )
                        wvts.append(wvt)
                    for ti in range(TC):
                        vps = ps.tile([128, 512], F32, tag="ps", name="vps")
                        for dc in range(DC):
                            nc.tensor.matmul(
                                vps[:],
                                xt[dc][:, ti * 128 : (ti + 1) * 128],
                                wvts[dc][:],
                                start=(dc == 0), stop=(dc == DC - 1),
                            )
                        # vq[ti] = [v*km | km] per head: [128, 8, 65] bf16
                        dst = vq[ti].rearrange("p (h c) -> p h c", c=65)
                        src3 = vps[:].rearrange("p (h c) -> p h c", c=64)
                        nc.vector.tensor_scalar_mul(dst[:, :, 0:64], src3, km[:, ti : ti + 1])
                        for l in range(8):
                            nc.vector.tensor_copy(dst[:, l, 64:65], km[:, ti : ti + 1])

                # kT2 / qT2 for this pair
                kps = [ps.tile([128, 512], F32, tag="ps", name=f"kps{j}") for j in range(4)]
                qps = [ps.tile([128, 512], F32, tag="ps", name=f"qps{j}") for j in range(2)]
                for dc in range(DC):
                    wkt = wst.tile([128, 128], F32R, tag="wst", name="wkt")
                    nc.sync.dma_start(out=wkt[:], in_=r32(wk[dc * 128 : (dc + 1) * 128, p * 128 : (p + 1) * 128]))
                    wqt = wst.tile([128, 128], F32R, tag="wst", name="wqt")
                    nc.sync.dma_start(out=wqt[:], in_=r32(wq[dc * 128 : (dc + 1) * 128, p * 128 : (p + 1) * 128]))
                    for j in range(4):
                        nc.tensor.matmul(kps[j][:], wkt[:], xt[dc][:, j * 512 : (j + 1) * 512],
                                         start=(dc == 0), stop=(dc == DC - 1))
                    for j in range(2):
                        nc.tensor.matmul(qps[j][:], wqt[:], xt[dc][:, j * 512 : (j + 1) * 512],
                                         start=(dc == 0), stop=(dc == DC - 1))
                kt2 = kt2p.tile([128, S], F32R, tag="kt2")
                for j in range(4):
                    nc.vector.tensor_copy(kt2[:, j * 512 : (j + 1) * 512], kps[j][:])
                qt2 = qt2p.tile([128, SQ], F32R, tag="qt2")
                for j in range(2):
                    nc.vector.tensor_copy(qt2[:, j * 512 : (j + 1) * 512], qps[j][:])

                if pending is not None:
                    emit_outproj(*pending)
                otn = otnp.tile([128, 1024], F32R, tag="otn")
                for h in range(2):
                    l = (p % 4) * 2 + h  # head slot within current vq half-round
                    oT = [ps.tile([128, 512], F32, tag="ps", name=f"oT{n2}") for n2 in range(2)]
                    prev_ex = None
                    for ti in range(TC):
                        exs = []
                        for n2 in range(2):
                            sc = ps.tile([128, 512], F32, tag="ps", name="sc")
                            nc.tensor.matmul(
                                sc[:],
                                kt2[h * 64 : (h + 1) * 64, ti * 128 : (ti + 1) * 128],
                                qt2[h * 64 : (h + 1) * 64, HALVES[n2]],
                                start=True, stop=True,
                            )
                            ex = expp.tile([128, 512], BF16, tag="exp", name="ex")
                            nc.scalar.activation(ex[:], sc[:], EXP, scale=SCALE)
                            exs.append(ex)
                        if prev_ex is not None:
                            pti, pex = prev_ex
                            for n2 in range(2):
                                nc.tensor.matmul(
                                    oT[n2][0:65, :],
                                    vq[pti][:, l * 65 : (l + 1) * 65],
                                    pex[n2][:],
                                    start=(pti == 0), stop=False,
                                )
                        prev_ex = (ti, exs)
                    pti, pex = prev_ex
                    for n2 in range(2):
                        nc.tensor.matmul(
                            oT[n2][0:65, :],
                            vq[pti][:, l * 65 : (l + 1) * 65],
                            pex[n2][:],
                            start=False, stop=True,
                        )
                    # normalize rows: o/denom, denom += 1e30 on masked queries
                    rsq = sm.tile([1, 1024], F32R, tag="rsq")
                    for n2 in range(2):
                        nc.vector.tensor_add(rsq[:, HALVES[n2]], oT[n2][64:65, :], qm[:, HALVES[n2]])
                    recB = tmp.tile([128, 1024], F32, tag="tmp")
                    for n2 in range(2):
                        rB = ps.tile([128, 512], F32, tag="ps", name="rB")
                        nc.tensor.matmul(rB[0:64, :], ones1[:, 0:64], rsq[:, HALVES[n2]],
                                         start=True, stop=True)
                        nc.vector.reciprocal_approx_fast(out=recB[0:64, HALVES[n2]], in_=rB[0:64, :])
                    for n2 in range(2):
                        nc.vector.tensor_mul(otn[h * 64 : (h + 1) * 64, HALVES[n2]],
                                             oT[n2][0:64, :], recB[0:64, HALVES[n2]])
                pending = (p, otn)
            emit_outproj(*pending)

            # ---- P2: saT -> yT in place; keep rmsy for the final residual ----
            ssy = [ps.tile([4, 512], F32, tag="ps", name=f"ssy{n2}") for n2 in range(2)]
            for dc in range(DC):
                sqy = tmp.tile([128, 1024], F32R, tag="tmp")
                nc.vector.tensor_mul(sqy[:], sat[dc][:], sat[dc][:])
                for n2 in range(2):
                    nc.tensor.matmul(ssy[n2][:], ones4[:], sqy[:, HALVES[n2]],
                                     start=(dc == 0), stop=(dc == DC - 1))
            rmsy = consts.tile([1, 1024], F32R, tag="rmsy")
            for n2 in range(2):
                nc.scalar.activation(rmsy[:, HALVES[n2]], ssy[n2][0:1, :], SQRT, scale=1.0 / D)
            invyB = tmp.tile([128, 1024], F32, tag="tmp")
            for n2 in range(2):
                rmsyB2 = ps.tile([128, 512], F32, tag="ps", name="rmsyB2")
                nc.tensor.matmul(rmsyB2[:], ones1[:], rmsy[:, HALVES[n2]], start=True, stop=True)
                nc.vector.reciprocal_approx_fast(out=invyB[:, HALVES[n2]], in_=rmsyB2[:])
            for dc in range(DC):
                nc.vector.tensor_mul(sat[dc][:], sat[dc][:], invyB[:])

            # ---- P3: h1 = silu(yT@W1)*(yT@V1) -> bf16, overlaid on xt space ----
            h1c = []
            for fc in range(FC):
                h1c.append(h1t[fc // 4][:, (fc % 4) * 1024 : (fc % 4 + 1) * 1024])
            for fc in range(FC):
                h1w = [ps.tile([128, 512], F32, tag="ps", name=f"h1w{n2}") for n2 in range(2)]
                h1v = [ps.tile([128, 512], F32, tag="ps", name=f"h1v{n2}") for n2 in range(2)]
                for dc in range(DC):
                    w1t = wst.tile([128, 128], F32R, tag="wst", name="w1t")
                    nc.sync.dma_start(out=w1t[:], in_=r32(w1[dc * 128 : (dc + 1) * 128, fc * 128 : (fc + 1) * 128]))
                    v1t = wst.tile([128, 128], F32R, tag="wst", name="v1t")
                    nc.sync.dma_start(out=v1t[:], in_=r32(v1[dc * 128 : (dc + 1) * 128, fc * 128 : (fc + 1) * 128]))
                    for n2 in range(2):
                        nc.tensor.matmul(h1w[n2][:], w1t[:], sat[dc][:, HALVES[n2]],
                                         start=(dc == 0), stop=(dc == DC - 1))
                        nc.tensor.matmul(h1v[n2][:], v1t[:], sat[dc][:, HALVES[n2]],
                                         start=(dc == 0), stop=(dc == DC - 1))
                sil = tmp.tile([128, 1024], F32, tag="tmp")
                for n2 in range(2):
                    nc.scalar.activation(sil[:, HALVES[n2]], h1w[n2][:], SILU)
                    nc.vector.tensor_mul(h1c[fc][:, HALVES[n2]], sil[:, HALVES[n2]], h1v[n2][:])

            # ---- P4: outT = yT*rmsy + h1 @ W2   (saT reconstructed) ----
            rmsyB = [ps.tile([128, 512], F32, tag="ps", name=f"rmsyB{n2}") for n2 in range(2)]
            for n2 in range(2):
                nc.tensor.matmul(rmsyB[n2][:], ones1[:], rmsy[:, HALVES[n2]], start=True, stop=True)
            for dc2 in range(DC):
                h2 = [ps.tile([128, 512], F32, tag="ps", name=f"h2{n2}") for n2 in range(2)]
                for fc in range(FC):
                    w2t = w2st.tile([128, 128], BF16, tag="w2st")
                    nc.sync.dma_start(out=w2t[:], in_=w2b[fc * 128 : (fc + 1) * 128, dc2 * 128 : (dc2 + 1) * 128])
                    for n2 in range(2):
                        nc.tensor.matmul(h2[n2][:], w2t[:], h1c[fc][:, HALVES[n2]],
                                         start=(fc == 0), stop=(fc == FC - 1))
                ot = tmp.tile([128, 1024], F32, tag="tmp")
                sa_rec = tmp.tile([128, 1024], F32, tag="tmp")
                for n2 in range(2):
                    nc.vector.tensor_mul(sa_rec[:, HALVES[n2]], sat[dc2][:, HALVES[n2]], rmsyB[n2][:])
                    nc.vector.tensor_add(ot[:, HALVES[n2]], sa_rec[:, HALVES[n2]], h2[n2][:])
                nc.sync.dma_start(out=outT[dc2 * 128 : (dc2 + 1) * 128, :], in_=ot[:])

    nc.compile()
    return nc


_NC = None


def _get_nc():
    global _NC
    if _NC is None:
        _NC = build()
    return _NC


def _build_in_maps(inputs):
    return _prep(**inputs)


def kernel(**inputs):
    in_maps = _prep(**inputs)
    res = run_bass_kernel_spmd(_get_nc(), in_maps, list(range(8)))
    out = np.empty((B, S, D), np.float32)
    for c in range(8):
        b, qh = c // 2, c % 2
        out[b, qh * SQ : (qh + 1) * SQ, :] = res.results[c]["outT"].T
    return out


def _prep(src, src_padding_mask, Wq, Wk, Wv, Wo, g1, g2, W1, V1, W2, **_):
    src = np.asarray(src, np.float32)
    valid = (~np.asarray(src_padding_mask, bool)).astype(np.float32)
    g1 = np.asarray(g1, np.float32)
    g2 = np.asarray(g2, np.float32)
    wq_cat = (np.transpose(np.asarray(Wq, np.float32), (1, 0, 2)).reshape(D, D)
              * g1[:, None]).astype(np.float32)
    wk_cat = (np.transpose(np.asarray(Wk, np.float32), (1, 0, 2)).reshape(D, D)
              * g1[:, None]).astype(np.float32)
    wv_cat = (np.transpose(np.asarray(Wv, np.float32), (1, 0, 2)).reshape(D, D)
              * g1[:, None]).astype(np.float32)
    wo_a = np.ascontiguousarray(np.asarray(Wo, np.float32))
    w1_s = np.ascontiguousarray(np.asarray(W1, np.float32) * g2[:, None])
    v1_s = np.ascontiguousarray(np.asarray(V1, np.float32) * g2[:, None])
    w2_b = np.asarray(W2, np.float32).astype(ml_dtypes.bfloat16)

    in_maps = []
    for c in range(8):
        b, qh = c // 2, c % 2
        roll = qh * SQ
        src_r = np.roll(src[b], -roll, axis=0)          # [S, D]
        srcT_c = np.ascontiguousarray(src_r.T)          # [D, S]
        km_c = np.ascontiguousarray(np.roll(valid[b], -roll)).reshape(S, 1)
        # additive softmax-denominator bias: +1e30 on padded query rows so
        # 1/denom ~ 0 there (reference zeroes those attention rows)
        qm_c = np.ascontiguousarray(((1.0 - km_c[0:SQ]) * 1e30).reshape(1, SQ))
        in_maps.append({
            "srcT": srcT_c, "kmask": km_c, "qmask": qm_c,
            "wq": wq_cat, "wk": wk_cat, "wv": wv_cat, "wo": wo_a,
            "w1": w1_s, "v1": v1_s, "w2b": w2_b,
        })
    return in_maps


# revision 30
# speedup vs baseline: 1.3017x; 1.1830x over previous
"""Trainium2 Bass kernel for a padded-attention transformer encoder layer.

Shapes (hardcoded): src [4, 2048, 1024], 16 heads x 64, d_ff 4096, 8 cores.

Sharding: each core computes the full layer for 1024 output tokens
(batch = core//2, token half = core%2). Inputs are host-transposed
(feature-major) and host-rolled so every core's query tokens are columns
0:1024 of its srcT; attention over keys is permutation invariant so the
roll only permutes the contraction order.

On-core dataflow (feature-on-partitions, fp32r matmuls, PSUM in 1-bank
[128,512] slots for deep PE run-ahead):
  xT = rmsnorm_T(srcT)                    (partition-dim reduce via ones-matmul)
  qT2/kT2 per head pair from xT; v (token-major, bf16) via xT-stationary matmuls
  scoresT[t,s] = kT.T @ qT (fp32r); expT = exp(scores/8) in bf16
  v_aug = [v*kmask | kmask] -> attnV (bf16) yields o and softmax denom at once
  o normalized by 1/(denom + 1e30*qpad), out-proj accumulated into saT
  yT = rmsnorm_T(saT) in place; h1 = silu(yT@W1)*(yT@V1) bf16 over dead xT space
  outT = yT*rms + h1 @ W2  (bf16 matmul, fp32 accum)
"""

import sys

sys.path.insert(0, "/opt/trn_rl_repo")

import numpy as np
import ml_dtypes

import concourse.bass as bass
import concourse.mybir as mybir
import concourse.tile as tile
from concourse import bacc
from concourse.bass_utils import run_bass_kernel_spmd

F32 = mybir.dt.float32
F32R = mybir.dt.float32r
BF16 = mybir.dt.bfloat16
EXP = mybir.ActivationFunctionType.Exp
SILU = mybir.ActivationFunctionType.Silu
SQRT = mybir.ActivationFunctionType.Sqrt

B, S, D, H, DK, DFF = 4, 2048, 1024, 16, 64, 4096
SQ = 1024          # query tokens per core
DC = D // 128      # 8 d-chunks
TC = S // 128      # 16 token chunks
FC = DFF // 128    # 32 dff chunks
NPAIR = H // 2     # 8 head pairs
SCALE = DK ** -0.5
H0, H1 = slice(0, 512), slice(512, 1024)
HALVES = (H0, H1)


def r32(ap):
    return ap.bitcast(F32R)


def build():
    nc = bacc.Bacc("TRN2", target_bir_lowering=False, debug=False, num_devices=8)

    srcT = nc.dram_tensor("srcT", [D, S], F32, kind="ExternalInput").ap()
    kmask = nc.dram_tensor("kmask", [S, 1], F32, kind="ExternalInput").ap()
    qmaskd = nc.dram_tensor("qmask", [1, SQ], F32, kind="ExternalInput").ap()
    wq = nc.dram_tensor("wq", [D, D], F32, kind="ExternalInput").ap()
    wk = nc.dram_tensor("wk", [D, D], F32, kind="ExternalInput").ap()
    wv = nc.dram_tensor("wv", [D, D], F32, kind="ExternalInput").ap()
    wo = nc.dram_tensor("wo", [D, D], F32, kind="ExternalInput").ap()
    w1 = nc.dram_tensor("w1", [D, DFF], F32, kind="ExternalInput").ap()
    v1 = nc.dram_tensor("v1", [D, DFF], F32, kind="ExternalInput").ap()
    w2b = nc.dram_tensor("w2b", [DFF, D], BF16, kind="ExternalInput").ap()
    outT = nc.dram_tensor("outT", [D, SQ], F32, kind="ExternalOutput").ap()

    # persistent SBUF arrays. h1 (bf16, FFN intermediate) aliases xt's bytes:
    # xt is fully consumed before the first h1 write (guaranteed through the
    # tracked saT dependency chain) and the aliased tensors keep the fp32r-
    # and bf16-consumed memory locations distinct for the BIR verifier.
    xt, h1t = [], []
    for i in range(DC):
        xt.append(nc.alloc_sbuf_tensor(f"xt{i}", [128, S], F32R).ap())
        off = nc.sbuf_base - S * 4
        h1t.append(nc.alloc_sbuf_tensor_at(f"h1t{i}", [128, 2 * S], BF16, offset=off).ap())
    # saT: residual stream, scaled in place to yT; residual restored as yT*rms
    sat = [nc.alloc_sbuf_tensor(f"sat{i}", [128, SQ], F32R).ap() for i in range(DC)]
    # v for one half-round (4 pairs = 8 heads), bf16, kmask-augmented col 64
    vq = [nc.alloc_sbuf_tensor(f"vq{i}", [128, 8 * 65], BF16).ap() for i in range(TC)]

    with nc.allow_low_precision(reason="fp32r/bf16 matmul operand rounding; fp32 PSUM accumulation"), \
         tile.TileContext(nc) as tc:
        with (
            tc.tile_pool(name="kt2p", bufs=1) as kt2p,
            tc.tile_pool(name="qzp", bufs=2) as qzp,
            tc.tile_pool(name="expp", bufs=6) as expp,
            tc.tile_pool(name="otnp", bufs=2) as otnp,
            tc.tile_pool(name="tmp", bufs=3) as tmp,
            tc.tile_pool(name="wst", bufs=8) as wst,
            tc.tile_pool(name="w2st", bufs=4) as w2st,
            tc.tile_pool(name="wost", bufs=2) as wost,
            tc.tile_pool(name="consts", bufs=1) as consts,
            tc.tile_pool(name="sm", bufs=2) as sm,
            tc.tile_pool(name="smr", bufs=1) as smr,
            tc.tile_pool(name="ps", bufs=8, space="PSUM") as ps,
        ):
            # ---- constants ----
            onesf = consts.tile([128, 128], F32, tag="onesf")
            nc.vector.memset(onesf[:], 1.0)
            ones4 = consts.tile([128, 4], F32R, tag="ones4")
            nc.vector.tensor_copy(ones4[:], onesf[:, 0:4])
            ones1 = consts.tile([1, 128], F32R, tag="ones1")
            nc.vector.tensor_copy(ones1[:], onesf[0:1, :])
            km = consts.tile([128, TC], F32, tag="km")
            for ti in range(TC):
                nc.sync.dma_start(out=km[:, ti : ti + 1], in_=kmask[ti * 128 : (ti + 1) * 128, :])
            qm = consts.tile([1, SQ], F32, tag="qm")
            nc.sync.dma_start(out=qm[:], in_=qmaskd[:])

            # ---- P0: xT = rmsnorm_T(srcT), in two 1024-col halves ----
            for th in range(2):
                hs = slice(th * 1024, (th + 1) * 1024)
                ssq = [ps.tile([4, 512], F32, tag="ps", name=f"ssq{n2}") for n2 in range(2)]
                for dc in range(DC):
                    ld = tmp.tile([128, 1024], F32R, tag="tmp")
                    nc.sync.dma_start(out=ld[:], in_=r32(srcT[dc * 128 : (dc + 1) * 128, hs]))
                    sq = tmp.tile([128, 1024], F32R, tag="tmp")
                    nc.vector.tensor_mul(sq[:], ld[:], ld[:])
                    for n2 in range(2):
                        nc.tensor.matmul(ssq[n2][:], ones4[:], sq[:, HALVES[n2]],
                                         start=(dc == 0), stop=(dc == DC - 1))
                rms = smr.tile([1, 1024], F32R, tag="rms")
                for n2 in range(2):
                    nc.scalar.activation(rms[:, HALVES[n2]], ssq[n2][0:1, :], SQRT, scale=1.0 / D)
                rmsB = [ps.tile([128, 512], F32, tag="ps", name=f"rmsB{n2}") for n2 in range(2)]
                for n2 in range(2):
                    nc.tensor.matmul(rmsB[n2][:], ones1[:], rms[:, HALVES[n2]], start=True, stop=True)
                invB = tmp.tile([128, 1024], F32, tag="tmp")
                for n2 in range(2):
                    nc.vector.reciprocal_approx_fast(out=invB[:, HALVES[n2]], in_=rmsB[n2][:])
                for dc in range(DC):
                    ld2 = tmp.tile([128, 1024], F32R, tag="tmp")
                    nc.sync.dma_start(out=ld2[:], in_=r32(srcT[dc * 128 : (dc + 1) * 128, hs]))
                    nc.vector.tensor_mul(xt[dc][:, hs], ld2[:], invB[:])

            # ---- P1: attention ----
            # out-proj of pair p is emitted after pair p+1's kT2/qT2 matmuls so
            # the PE never stalls on the softmax-denominator reciprocal chain.
            def emit_outproj(p, otn):
                for dc2 in range(DC):
                    wot = wost.tile([128, 128], F32R, tag="wost", name="wot")
                    nc.sync.dma_start(out=wot[:], in_=r32(wo[p * 128 : (p + 1) * 128, dc2 * 128 : (dc2 + 1) * 128]))
                    for n2 in range(2):
                        pp = ps.tile([128, 512], F32, tag="ps", name="pp")
                        nc.tensor.matmul(pp[:], wot[:], otn[:, HALVES[n2]], start=True, stop=True)
                        if p == 0:
                            srcq = tmp.tile([128, 512], F32, tag="tmp", name="srcq")
                            nc.sync.dma_start(out=srcq[:], in_=srcT[dc2 * 128 : (dc2 + 1) * 128, th_sq(n2)])
                            nc.vector.tensor_add(sat[dc2][:, HALVES[n2]], pp[:], srcq[:])
                        else:
                            nc.vector.tensor_add(sat[dc2][:, HALVES[n2]], sat[dc2][:, HALVES[n2]], pp[:])

            def th_sq(n2):
                return slice(n2 * 512, (n2 + 1) * 512)

            pending = None  # (pair index, otn tile) awaiting out-proj
            for p in range(NPAIR):
                vr, lp = p // 4, p % 4
                if lp == 0:
                    # v for pairs 4vr..4vr+3 (8 heads, 512 wv cols), bf16+kmask aug
                    wvts = []
                    for dc in range(DC):
                        wvt = wst.tile([128, 512], F32R, tag="wst", name="wvt")
                        nc.sync.dma_start(
                            out=wvt[:],
                            in_=r32(wv[dc * 128 : (dc + 1) * 128, vr * 512 : (vr + 1) * 512]),
                        )
                        wvts.append(wvt)
                    for ti in range(TC):
                        vps = ps.tile([128, 512], F32, tag="ps", name="vps")
                        for dc in range(DC):
                            nc.tensor.matmul(
                                vps[:],
                                xt[dc][:, ti * 128 : (ti + 1) * 128],
                                wvts[dc][:],
                                start=(dc == 0), stop=(dc == DC - 1),
                            )
                        # vq[ti] = [v*km | km] per head: [128, 8, 65] bf16
                        dst = vq[ti].rearrange("p (h c) -> p h c", c=65)
                        src3 = vps[:].rearrange("p (h c) -> p h c", c=64)
                        nc.vector.tensor_scalar_mul(dst[:, :, 0:64], src3, km[:, ti : ti + 1])
                        for l in range(8):
                            nc.vector.tensor_copy(dst[:, l, 64:65], km[:, ti : ti + 1])

                # kT2 / qT2 for this pair
                kps = [ps.tile([128, 512], F32, tag="ps", name=f"kps{j}") for j in range(4)]
                qps = [ps.tile([128, 512], F32, tag="ps", name=f"qps{j}") for j in range(2)]
                for dc in range(DC):
                    wkt = wst.tile([128, 128], F32R, tag="wst", name="wkt")
                    nc.sync.dma_start(out=wkt[:], in_=r32(wk[dc * 128 : (dc + 1) * 128, p * 128 : (p + 1) * 128]))
                    wqt = wst.tile([128, 128], F32R, tag="wst", name="wqt")
                    nc.sync.dma_start(out=wqt[:], in_=r32(wq[dc * 128 : (dc + 1) * 128, p * 128 : (p + 1) * 128]))
                    for j in range(4):
                        nc.tensor.matmul(kps[j][:], wkt[:], xt[dc][:, j * 512 : (j + 1) * 512],
                                         start=(dc == 0), stop=(dc == DC - 1))
                    for j in range(2):
                        nc.tensor.matmul(qps[j][:], wqt[:], xt[dc][:, j * 512 : (j + 1) * 512],
                                         start=(dc == 0), stop=(dc == DC - 1))
                kt2 = kt2p.tile([128, S], F32R, tag="kt2")
                for j in range(4):
                    nc.vector.tensor_copy(kt2[:, j * 512 : (j + 1) * 512], kps[j][:])
                # zero-padded per-head q: scores contract at K=128 (full array)
                qzA = qzp.tile([128, SQ], F32R, tag="qzA")
                qzB = qzp.tile([128, SQ], F32R, tag="qzB")
                for j in range(2):
                    js = slice(j * 512, (j + 1) * 512)
                    nc.vector.tensor_copy(qzA[0:64, js], qps[j][0:64, :])
                    nc.vector.tensor_scalar_mul(qzA[64:128, js], qps[j][64:128, :], 0.0)
                    nc.vector.tensor_copy(qzB[64:128, js], qps[j][64:128, :])
                    nc.vector.tensor_scalar_mul(qzB[0:64, js], qps[j][0:64, :], 0.0)

                if pending is not None:
                    emit_outproj(*pending)
                otn = otnp.tile([128, 1024], F32R, tag="otn")
                for h in range(2):
                    l = (p % 4) * 2 + h  # head slot within current vq half-round
                    oT = [ps.tile([128, 512], F32, tag="ps", name=f"oT{n2}") for n2 in range(2)]
                    prev_ex = None
                    for ti in range(TC):
                        exs = []
                        for n2 in range(2):
                            sc = ps.tile([128, 512], F32, tag="ps", name="sc")
                            qz = qzA if h == 0 else qzB
                            nc.tensor.matmul(
                                sc[:],
                                kt2[:, ti * 128 : (ti + 1) * 128],
                                qz[:, HALVES[n2]],
                                start=True, stop=True,
                            )
                            ex = expp.tile([128, 512], BF16, tag="exp", name="ex")
                            nc.scalar.activation(ex[:], sc[:], EXP, scale=SCALE)
                            exs.append(ex)
                        if prev_ex is not None:
                            pti, pex = prev_ex
                            for n2 in range(2):
                                nc.tensor.matmul(
                                    oT[n2][0:65, :],
                                    vq[pti][:, l * 65 : (l + 1) * 65],
                                    pex[n2][:],
                                    start=(pti == 0), stop=False,
                                )
                        prev_ex = (ti, exs)
                    pti, pex = prev_ex
                    for n2 in range(2):
                        nc.tensor.matmul(
                            oT[n2][0:65, :],
                            vq[pti][:, l * 65 : (l + 1) * 65],
                            pex[n2][:],
                            start=False, stop=True,
                        )
                    # normalize rows: o/denom, denom += 1e30 on masked queries
                    rsq = sm.tile([1, 1024], F32R, tag="rsq")
                    for n2 in range(2):
                        nc.vector.tensor_add(rsq[:, HALVES[n2]], oT[n2][64:65, :], qm[:, HALVES[n2]])
                    recB = tmp.tile([128, 1024], F32, tag="tmp")
                    for n2 in range(2):
                        rB = ps.tile([128, 512], F32, tag="ps", name="rB")
                        nc.tensor.matmul(rB[0:64, :], ones1[:, 0:64], rsq[:, HALVES[n2]],
                                         start=True, stop=True)
                        nc.vector.reciprocal_approx_fast(out=recB[0:64, HALVES[n2]], in_=rB[0:64, :])
                    for n2 in range(2):
                        nc.vector.tensor_mul(otn[h * 64 : (h + 1) * 64, HALVES[n2]],
                                             oT[n2][0:64, :], recB[0:64, HALVES[n2]])
                pending = (p, otn)
            emit_outproj(*pending)

            # ---- P2: saT -> yT in place; keep rmsy for the final residual ----
            ssy = [ps.tile([4, 512], F32, tag="ps", name=f"ssy{n2}") for n2 in range(2)]
            for dc in range(DC):
                sqy = tmp.tile([128, 1024], F32R, tag="tmp")
                nc.vector.tensor_mul(sqy[:], sat[dc][:], sat[dc][:])
                for n2 in range(2):
                    nc.tensor.matmul(ssy[n2][:], ones4[:], sqy[:, HALVES[n2]],
                                     start=(dc == 0), stop=(dc == DC - 1))
            rmsy = consts.tile([1, 1024], F32R, tag="rmsy")
            for n2 in range(2):
                nc.scalar.activation(rmsy[:, HALVES[n2]], ssy[n2][0:1, :], SQRT, scale=1.0 / D)
            invyB = tmp.tile([128, 1024], F32, tag="tmp")
            for n2 in range(2):
                rmsyB2 = ps.tile([128, 512], F32, tag="ps", name="rmsyB2")
                nc.tensor.matmul(rmsyB2[:], ones1[:], rmsy[:, HALVES[n2]], start=True, stop=True)
                nc.vector.reciprocal_approx_fast(out=invyB[:, HALVES[n2]], in_=rmsyB2[:])
            for dc in range(DC):
                nc.vector.tensor_mul(sat[dc][:], sat[dc][:], invyB[:])

            # ---- P3: h1 = silu(yT@W1)*(yT@V1) -> bf16, overlaid on xt space ----
            h1c = []
            for fc in range(FC):
                h1c.append(h1t[fc // 4][:, (fc % 4) * 1024 : (fc % 4 + 1) * 1024])
            for fc in range(FC):
                h1w = [ps.tile([128, 512], F32, tag="ps", name=f"h1w{n2}") for n2 in range(2)]
                h1v = [ps.tile([128, 512], F32, tag="ps", name=f"h1v{n2}") for n2 in range(2)]
                for dc in range(DC):
                    w1t = wst.tile([128, 128], F32R, tag="wst", name="w1t")
                    nc.sync.dma_start(out=w1t[:], in_=r32(w1[dc * 128 : (dc + 1) * 128, fc * 128 : (fc + 1) * 128]))
                    v1t = wst.tile([128, 128], F32R, tag="wst", name="v1t")
                    nc.sync.dma_start(out=v1t[:], in_=r32(v1[dc * 128 : (dc + 1) * 128, fc * 128 : (fc + 1) * 128]))
                    for n2 in range(2):
                        nc.tensor.matmul(h1w[n2][:], w1t[:], sat[dc][:, HALVES[n2]],
                                         start=(dc == 0), stop=(dc == DC - 1))
                        nc.tensor.matmul(h1v[n2][:], v1t[:], sat[dc][:, HALVES[n2]],
                                         start=(dc == 0), stop=(dc == DC - 1))
                sil = tmp.tile([128, 1024], F32, tag="tmp")
                for n2 in range(2):
                    nc.scalar.activation(sil[:, HALVES[n2]], h1w[n2][:], SILU)
                    nc.vector.tensor_mul(h1c[fc][:, HALVES[n2]], sil[:, HALVES[n2]], h1v[n2][:])

            # ---- P4: outT = yT*rmsy + h1 @ W2   (saT reconstructed) ----
            rmsyB = [ps.tile([128, 512], F32, tag="ps", name=f"rmsyB{n2}") for n2 in range(2)]
            for n2 in range(2):
                nc.tensor.matmul(rmsyB[n2][:], ones1[:], rmsy[:, HALVES[n2]], start=True, stop=True)
            for dc2 in range(DC):
                h2 = [ps.tile([128, 512], F32, tag="ps", name=f"h2{n2}") for n2 in range(2)]
                for fc in range(FC):
                    w2t = w2st.tile([128, 128], BF16, tag="w2st")
                    nc.sync.dma_start(out=w2t[:], in_=w2b[fc * 128 : (fc + 1) * 128, dc2 * 128 : (dc2 + 1) * 128])
                    for n2 in range(2):
                        nc.tensor.matmul(h2[n2][:], w2t[:], h1c[fc][:, HALVES[n2]],
                                         start=(fc == 0), stop=(fc == FC - 1))
                ot = tmp.tile([128, 1024], F32, tag="tmp")
                sa_rec = tmp.tile([128, 1024], F32, tag="tmp")
                for n2 in range(2):
                    nc.vector.tensor_mul(sa_rec[:, HALVES[n2]], sat[dc2][:, HALVES[n2]], rmsyB[n2][:])
                    nc.vector.tensor_add(ot[:, HALVES[n2]], sa_rec[:, HALVES[n2]], h2[n2][:])
                nc.sync.dma_start(out=outT[dc2 * 128 : (dc2 + 1) * 128, :], in_=ot[:])

    nc.compile()
    return nc


_NC = None


def _get_nc():
    global _NC
    if _NC is None:
        _NC = build()
    return _NC


def _build_in_maps(inputs):
    return _prep(**inputs)


def kernel(**inputs):
    in_maps = _prep(**inputs)
    res = run_bass_kernel_spmd(_get_nc(), in_maps, list(range(8)))
    out = np.empty((B, S, D), np.float32)
    for c in range(8):
        b, qh = c // 2, c % 2
        out[b, qh * SQ : (qh + 1) * SQ, :] = res.results[c]["outT"].T
    return out


def _prep(src, src_padding_mask, Wq, Wk, Wv, Wo, g1, g2, W1, V1, W2, **_):
    src = np.asarray(src, np.float32)
    valid = (~np.asarray(src_padding_mask, bool)).astype(np.float32)
    g1 = np.asarray(g1, np.float32)
    g2 = np.asarray(g2, np.float32)
    wq_cat = (np.transpose(np.asarray(Wq, np.float32), (1, 0, 2)).reshape(D, D)
              * g1[:, None]).astype(np.float32)
    wk_cat = (np.transpose(np.asarray(Wk, np.float32), (1, 0, 2)).reshape(D, D)
              * g1[:, None]).astype(np.float32)
    wv_cat = (np.transpose(np.asarray(Wv, np.float32), (1, 0, 2)).reshape(D, D)
              * g1[:, None]).astype(np.float32)
    wo_a = np.ascontiguousarray(np.asarray(Wo, np.float32))
    w1_s = np.ascontiguousarray(np.asarray(W1, np.float32) * g2[:, None])
    v1_s = np.ascontiguousarray(np.asarray(V1, np.float32) * g2[:, None])
    w2_b = np.asarray(W2, np.float32).astype(ml_dtypes.bfloat16)

    in_maps = []
    for c in range(8):
        b, qh = c // 2, c % 2
        roll = qh * SQ
        src_r = np.roll(src[b], -roll, axis=0)          # [S, D]
        srcT_c = np.ascontiguousarray(src_r.T)          # [D, S]
        km_c = np.ascontiguousarray(np.roll(valid[b], -roll)).reshape(S, 1)
        # additive softmax-denominator bias: +1e30 on padded query rows so
        # 1/denom ~ 0 there (reference zeroes those attention rows)
        qm_c = np.ascontiguousarray(((1.0 - km_c[0:SQ]) * 1e30).reshape(1, SQ))
        in_maps.append({
            "srcT": srcT_c, "kmask": km_c, "qmask": qm_c,
            "wq": wq_cat, "wk": wk_cat, "wv": wv_cat, "wo": wo_a,
            "w1": w1_s, "v1": v1_s, "w2b": w2_b,
        })
    return in_maps


# revision 32
# speedup vs baseline: 1.4382x; 1.1048x over previous
"""Trainium2 Bass kernel for a padded-attention transformer encoder layer.

Shapes (hardcoded): src [4, 2048, 1024], 16 heads x 64, d_ff 4096, 8 cores.

Sharding: each core computes the full layer for 1024 output tokens
(batch = core//2, token half = core%2). Inputs are host-transposed
(feature-major) and host-rolled so every core's query tokens are columns
0:1024 of its srcT; attention over keys is permutation invariant so the
roll only permutes the contraction order.

On-core dataflow (feature-on-partitions, fp32r matmuls, PSUM in 1-bank
[128,512] slots for deep PE run-ahead):
  xT = rmsnorm_T(srcT)                    (partition-dim reduce via ones-matmul)
  qT2/kT2 per head pair from xT; v (token-major, bf16) via xT-stationary matmuls
  scoresT[t,s] = kT.T @ qT (fp32r); expT = exp(scores/8) in bf16
  v_aug = [v*kmask | kmask] -> attnV (bf16) yields o and softmax denom at once
  o normalized by 1/(denom + 1e30*qpad), out-proj accumulated into saT
  yT = rmsnorm_T(saT) in place; h1 = silu(yT@W1)*(yT@V1) bf16 over dead xT space
  outT = yT*rms + h1 @ W2  (bf16 matmul, fp32 accum)
"""

import sys

sys.path.insert(0, "/opt/trn_rl_repo")

import numpy as np
import ml_dtypes

import concourse.bass as bass
import concourse.mybir as mybir
import concourse.tile as tile
from concourse import bacc
from concourse.bass_utils import run_bass_kernel_spmd

F32 = mybir.dt.float32
F32R = mybir.dt.float32r
BF16 = mybir.dt.bfloat16
EXP = mybir.ActivationFunctionType.Exp
SILU = mybir.ActivationFunctionType.Silu
SQRT = mybir.ActivationFunctionType.Sqrt

B, S, D, H, DK, DFF = 4, 2048, 1024, 16, 64, 4096
SQ = 1024          # query tokens per core
DC = D // 128      # 8 d-chunks
TC = S // 128      # 16 token chunks
FC = DFF // 128    # 32 dff chunks
NPAIR = H // 2     # 8 head pairs
SCALE = DK ** -0.5
H0, H1 = slice(0, 512), slice(512, 1024)
HALVES = (H0, H1)


def r32(ap):
    return ap.bitcast(F32R)


def build():
    nc = bacc.Bacc("TRN2", target_bir_lowering=False, debug=False, num_devices=8)

    srcT = nc.dram_tensor("srcT", [D, S], F32, kind="ExternalInput").ap()
    kmask = nc.dram_tensor("kmask", [S, 1], F32, kind="ExternalInput").ap()
    qmaskd = nc.dram_tensor("qmask", [1, SQ], F32, kind="ExternalInput").ap()
    wq = nc.dram_tensor("wq", [D, D], F32, kind="ExternalInput").ap()
    wk = nc.dram_tensor("wk", [D, D], F32, kind="ExternalInput").ap()
    wv = nc.dram_tensor("wv", [D, D], F32, kind="ExternalInput").ap()
    wo = nc.dram_tensor("wo", [D, D], F32, kind="ExternalInput").ap()
    w1 = nc.dram_tensor("w1", [D, DFF], F32, kind="ExternalInput").ap()
    v1 = nc.dram_tensor("v1", [D, DFF], F32, kind="ExternalInput").ap()
    w2b = nc.dram_tensor("w2b", [DFF, D], BF16, kind="ExternalInput").ap()
    outT = nc.dram_tensor("outT", [D, SQ], F32, kind="ExternalOutput").ap()

    # persistent SBUF arrays. h1 (bf16, FFN intermediate) aliases xt's bytes:
    # xt is fully consumed before the first h1 write (guaranteed through the
    # tracked saT dependency chain) and the aliased tensors keep the fp32r-
    # and bf16-consumed memory locations distinct for the BIR verifier.
    xt, h1t = [], []
    for i in range(DC):
        xt.append(nc.alloc_sbuf_tensor(f"xt{i}", [128, S], F32R).ap())
        off = nc.sbuf_base - S * 4
        h1t.append(nc.alloc_sbuf_tensor_at(f"h1t{i}", [128, 2 * S], BF16, offset=off).ap())
    # saT: residual stream, scaled in place to yT; residual restored as yT*rms
    sat = [nc.alloc_sbuf_tensor(f"sat{i}", [128, SQ], F32R).ap() for i in range(DC)]
    # v for one half-round (4 pairs = 8 heads), bf16, kmask-augmented col 64
    vq = [nc.alloc_sbuf_tensor(f"vq{i}", [128, 8 * 65], BF16).ap() for i in range(TC)]

    with nc.allow_low_precision(reason="fp32r/bf16 matmul operand rounding; fp32 PSUM accumulation"), \
         tile.TileContext(nc) as tc:
        with (
            tc.tile_pool(name="kt2p", bufs=1) as kt2p,
            tc.tile_pool(name="qzp", bufs=2) as qzp,
            tc.tile_pool(name="expp", bufs=6) as expp,
            tc.tile_pool(name="otnp", bufs=2) as otnp,
            tc.tile_pool(name="tmp", bufs=3) as tmp,
            tc.tile_pool(name="wst", bufs=8) as wst,
            tc.tile_pool(name="w2st", bufs=4) as w2st,
            tc.tile_pool(name="wost", bufs=2) as wost,
            tc.tile_pool(name="consts", bufs=1) as consts,
            tc.tile_pool(name="sm", bufs=2) as sm,
            tc.tile_pool(name="smr", bufs=1) as smr,
            tc.tile_pool(name="ps", bufs=8, space="PSUM") as ps,
        ):
            # ---- constants ----
            onesf = consts.tile([128, 128], F32, tag="onesf")
            nc.vector.memset(onesf[:], 1.0)
            ones4 = consts.tile([128, 4], F32R, tag="ones4")
            nc.vector.tensor_copy(ones4[:], onesf[:, 0:4])
            ones1 = consts.tile([1, 128], F32R, tag="ones1")
            nc.vector.tensor_copy(ones1[:], onesf[0:1, :])
            km = consts.tile([128, TC], F32, tag="km")
            for ti in range(TC):
                nc.sync.dma_start(out=km[:, ti : ti + 1], in_=kmask[ti * 128 : (ti + 1) * 128, :])
            qm = consts.tile([1, SQ], F32, tag="qm")
            nc.sync.dma_start(out=qm[:], in_=qmaskd[:])

            def vround_weights(vr):
                wvts = []
                for dc in range(DC):
                    wvt = wst.tile([128, 512], F32R, tag="wst", name="wvt")
                    nc.sync.dma_start(
                        out=wvt[:],
                        in_=r32(wv[dc * 128 : (dc + 1) * 128, vr * 512 : (vr + 1) * 512]),
                    )
                    wvts.append(wvt)
                return wvts

            def emit_vround(wvts, tis):
                for ti in tis:
                    vps = ps.tile([128, 512], F32, tag="ps", name="vps")
                    for dc in range(DC):
                        nc.tensor.matmul(
                            vps[:],
                            xt[dc][:, ti * 128 : (ti + 1) * 128],
                            wvts[dc][:],
                            start=(dc == 0), stop=(dc == DC - 1),
                        )
                    # vq[ti] = [v*km | km] per head: [128, 8, 65] bf16
                    dst = vq[ti].rearrange("p (h c) -> p h c", c=65)
                    src3 = vps[:].rearrange("p (h c) -> p h c", c=64)
                    nc.vector.tensor_scalar_mul(dst[:, :, 0:64], src3, km[:, ti : ti + 1])
                    for l in range(8):
                        nc.vector.tensor_copy(dst[:, l, 64:65], km[:, ti : ti + 1])

            # ---- P0: xT = rmsnorm_T(srcT), in two 1024-col halves; the first
            # v half-round runs on each xT half as it completes to keep PE busy
            wvts0 = vround_weights(0)
            for th in range(2):
                hs = slice(th * 1024, (th + 1) * 1024)
                ssq = [ps.tile([4, 512], F32, tag="ps", name=f"ssq{n2}") for n2 in range(2)]
                for dc in range(DC):
                    ld = tmp.tile([128, 1024], F32R, tag="tmp")
                    nc.sync.dma_start(out=ld[:], in_=r32(srcT[dc * 128 : (dc + 1) * 128, hs]))
                    sq = tmp.tile([128, 1024], F32R, tag="tmp")
                    nc.vector.tensor_mul(sq[:], ld[:], ld[:])
                    for n2 in range(2):
                        nc.tensor.matmul(ssq[n2][:], ones4[:], sq[:, HALVES[n2]],
                                         start=(dc == 0), stop=(dc == DC - 1))
                rms = smr.tile([1, 1024], F32R, tag="rms")
                for n2 in range(2):
                    nc.scalar.activation(rms[:, HALVES[n2]], ssq[n2][0:1, :], SQRT, scale=1.0 / D)
                rmsB = [ps.tile([128, 512], F32, tag="ps", name=f"rmsB{n2}") for n2 in range(2)]
                for n2 in range(2):
                    nc.tensor.matmul(rmsB[n2][:], ones1[:], rms[:, HALVES[n2]], start=True, stop=True)
                invB = tmp.tile([128, 1024], F32, tag="tmp")
                for n2 in range(2):
                    nc.vector.reciprocal_approx_fast(out=invB[:, HALVES[n2]], in_=rmsB[n2][:])
                for dc in range(DC):
                    ld2 = tmp.tile([128, 1024], F32R, tag="tmp")
                    nc.sync.dma_start(out=ld2[:], in_=r32(srcT[dc * 128 : (dc + 1) * 128, hs]))
                    nc.vector.tensor_mul(xt[dc][:, hs], ld2[:], invB[:])
                emit_vround(wvts0, range(th * 8, (th + 1) * 8))

            # ---- P1: attention ----
            # out-proj of pair p is emitted after pair p+1's kT2/qT2 matmuls so
            # the PE never stalls on the softmax-denominator reciprocal chain.
            def emit_outproj(p, otn):
                for dc2 in range(DC):
                    wot = wost.tile([128, 128], F32R, tag="wost", name="wot")
                    nc.sync.dma_start(out=wot[:], in_=r32(wo[p * 128 : (p + 1) * 128, dc2 * 128 : (dc2 + 1) * 128]))
                    for n2 in range(2):
                        pp = ps.tile([128, 512], F32, tag="ps", name="pp")
                        nc.tensor.matmul(pp[:], wot[:], otn[:, HALVES[n2]], start=True, stop=True)
                        if p == 0:
                            srcq = tmp.tile([128, 512], F32, tag="tmp", name="srcq")
                            nc.sync.dma_start(out=srcq[:], in_=srcT[dc2 * 128 : (dc2 + 1) * 128, th_sq(n2)])
                            nc.vector.tensor_add(sat[dc2][:, HALVES[n2]], pp[:], srcq[:])
                        else:
                            nc.vector.tensor_add(sat[dc2][:, HALVES[n2]], sat[dc2][:, HALVES[n2]], pp[:])

            def th_sq(n2):
                return slice(n2 * 512, (n2 + 1) * 512)

            pending = None  # (pair index, otn tile) awaiting out-proj
            for p in range(NPAIR):
                vr, lp = p // 4, p % 4
                if p == 4:
                    wvts1 = vround_weights(1)
                    emit_vround(wvts1, range(TC))

                # kT2 / qT2 for this pair
                kps = [ps.tile([128, 512], F32, tag="ps", name=f"kps{j}") for j in range(4)]
                for dc in range(DC):
                    wkt = wst.tile([128, 128], F32R, tag="wst", name="wkt")
                    nc.sync.dma_start(out=wkt[:], in_=r32(wk[dc * 128 : (dc + 1) * 128, p * 128 : (p + 1) * 128]))
                    for j in range(4):
                        nc.tensor.matmul(kps[j][:], wkt[:], xt[dc][:, j * 512 : (j + 1) * 512],
                                         start=(dc == 0), stop=(dc == DC - 1))
                kt2 = kt2p.tile([128, S], F32R, tag="kt2")
                for j in range(4):
                    nc.vector.tensor_copy(kt2[:, j * 512 : (j + 1) * 512], kps[j][:])
                qps = [ps.tile([128, 512], F32, tag="ps", name=f"qps{j}") for j in range(2)]
                for dc in range(DC):
                    wqt = wst.tile([128, 128], F32R, tag="wst", name="wqt")
                    nc.sync.dma_start(out=wqt[:], in_=r32(wq[dc * 128 : (dc + 1) * 128, p * 128 : (p + 1) * 128]))
                    for j in range(2):
                        nc.tensor.matmul(qps[j][:], wqt[:], xt[dc][:, j * 512 : (j + 1) * 512],
                                         start=(dc == 0), stop=(dc == DC - 1))
                # zero-padded per-head q: scores contract at K=128 (full array)
                qzA = qzp.tile([128, SQ], F32R, tag="qzA")
                qzB = qzp.tile([128, SQ], F32R, tag="qzB")
                for j in range(2):
                    js = slice(j * 512, (j + 1) * 512)
                    nc.vector.tensor_copy(qzA[0:64, js], qps[j][0:64, :])
                    nc.vector.tensor_scalar_mul(qzA[64:128, js], qps[j][64:128, :], 0.0)
                    nc.vector.tensor_copy(qzB[64:128, js], qps[j][64:128, :])
                    nc.vector.tensor_scalar_mul(qzB[0:64, js], qps[j][0:64, :], 0.0)

                if pending is not None:
                    emit_outproj(*pending)
                otn = otnp.tile([128, 1024], F32R, tag="otn")
                for h in range(2):
                    l = (p % 4) * 2 + h  # head slot within current vq half-round
                    oT = [ps.tile([128, 512], F32, tag="ps", name=f"oT{n2}") for n2 in range(2)]
                    prev_ex = None
                    for ti in range(TC):
                        exs = []
                        for n2 in range(2):
                            sc = ps.tile([128, 512], F32, tag="ps", name="sc")
                            qz = qzA if h == 0 else qzB
                            nc.tensor.matmul(
                                sc[:],
                                kt2[:, ti * 128 : (ti + 1) * 128],
                                qz[:, HALVES[n2]],
                                start=True, stop=True,
                            )
                            ex = expp.tile([128, 512], BF16, tag="exp", name="ex")
                            nc.scalar.activation(ex[:], sc[:], EXP, scale=SCALE)
                            exs.append(ex)
                        if prev_ex is not None:
                            pti, pex = prev_ex
                            for n2 in range(2):
                                nc.tensor.matmul(
                                    oT[n2][0:65, :],
                                    vq[pti][:, l * 65 : (l + 1) * 65],
                                    pex[n2][:],
                                    start=(pti == 0), stop=False,
                                )
                        prev_ex = (ti, exs)
                    pti, pex = prev_ex
                    for n2 in range(2):
                        nc.tensor.matmul(
                            oT[n2][0:65, :],
                            vq[pti][:, l * 65 : (l + 1) * 65],
                            pex[n2][:],
                            start=False, stop=True,
                        )
                    # normalize rows: o/denom, denom += 1e30 on masked queries
                    rsq = sm.tile([1, 1024], F32R, tag="rsq")
                    for n2 in range(2):
                        nc.vector.tensor_add(rsq[:, HALVES[n2]], oT[n2][64:65, :], qm[:, HALVES[n2]])
                    recB = tmp.tile([128, 1024], F32, tag="tmp")
                    for n2 in range(2):
                        rB = ps.tile([128, 512], F32, tag="ps", name="rB")
                        nc.tensor.matmul(rB[0:64, :], ones1[:, 0:64], rsq[:, HALVES[n2]],
                                         start=True, stop=True)
                        nc.vector.reciprocal_approx_fast(out=recB[0:64, HALVES[n2]], in_=rB[0:64, :])
                    for n2 in range(2):
                        nc.vector.tensor_mul(otn[h * 64 : (h + 1) * 64, HALVES[n2]],
                                             oT[n2][0:64, :], recB[0:64, HALVES[n2]])
                pending = (p, otn)
            emit_outproj(*pending)

            # ---- P2: saT -> yT in place; keep rmsy for the final residual ----
            ssy = [ps.tile([4, 512], F32, tag="ps", name=f"ssy{n2}") for n2 in range(2)]
            for dc in range(DC):
                sqy = tmp.tile([128, 1024], F32R, tag="tmp")
                nc.vector.tensor_mul(sqy[:], sat[dc][:], sat[dc][:])
                for n2 in range(2):
                    nc.tensor.matmul(ssy[n2][:], ones4[:], sqy[:, HALVES[n2]],
                                     start=(dc == 0), stop=(dc == DC - 1))
            rmsy = consts.tile([1, 1024], F32R, tag="rmsy")
            for n2 in range(2):
                nc.scalar.activation(rmsy[:, HALVES[n2]], ssy[n2][0:1, :], SQRT, scale=1.0 / D)
            invyB = tmp.tile([128, 1024], F32, tag="tmp")
            for n2 in range(2):
                rmsyB2 = ps.tile([128, 512], F32, tag="ps", name="rmsyB2")
                nc.tensor.matmul(rmsyB2[:], ones1[:], rmsy[:, HALVES[n2]], start=True, stop=True)
                nc.vector.reciprocal_approx_fast(out=invyB[:, HALVES[n2]], in_=rmsyB2[:])
            for dc in range(DC):
                nc.vector.tensor_mul(sat[dc][:], sat[dc][:], invyB[:])

            # ---- P3: h1 = silu(yT@W1)*(yT@V1) -> bf16, overlaid on xt space ----
            h1c = []
            for fc in range(FC):
                h1c.append(h1t[fc // 4][:, (fc % 4) * 1024 : (fc % 4 + 1) * 1024])
            for fc in range(FC):
                h1w = [ps.tile([128, 512], F32, tag="ps", name=f"h1w{n2}") for n2 in range(2)]
                h1v = [ps.tile([128, 512], F32, tag="ps", name=f"h1v{n2}") for n2 in range(2)]
                for dc in range(DC):
                    w1t = wst.tile([128, 128], F32R, tag="wst", name="w1t")
                    nc.sync.dma_start(out=w1t[:], in_=r32(w1[dc * 128 : (dc + 1) * 128, fc * 128 : (fc + 1) * 128]))
                    v1t = wst.tile([128, 128], F32R, tag="wst", name="v1t")
                    nc.scalar.dma_start(out=v1t[:], in_=r32(v1[dc * 128 : (dc + 1) * 128, fc * 128 : (fc + 1) * 128]))
                    for n2 in range(2):
                        nc.tensor.matmul(h1w[n2][:], w1t[:], sat[dc][:, HALVES[n2]],
                                         start=(dc == 0), stop=(dc == DC - 1))
                        nc.tensor.matmul(h1v[n2][:], v1t[:], sat[dc][:, HALVES[n2]],
                                         start=(dc == 0), stop=(dc == DC - 1))
                sil = tmp.tile([128, 1024], F32, tag="tmp")
                for n2 in range(2):
                    nc.scalar.activation(sil[:, HALVES[n2]], h1w[n2][:], SILU)
                    nc.vector.tensor_mul(h1c[fc][:, HALVES[n2]], sil[:, HALVES[n2]], h1v[n2][:])

            # ---- P4: outT = yT*rmsy + h1 @ W2   (saT reconstructed) ----
            rmsyB = [ps.tile([128, 512], F32, tag="ps", name=f"rmsyB{n2}") for n2 in range(2)]
            for n2 in range(2):
                nc.tensor.matmul(rmsyB[n2][:], ones1[:], rmsy[:, HALVES[n2]], start=True, stop=True)
            for dc2 in range(DC):
                h2 = [ps.tile([128, 512], F32, tag="ps", name=f"h2{n2}") for n2 in range(2)]
                for fc in range(FC):
                    w2t = w2st.tile([128, 128], BF16, tag="w2st")
                    nc.scalar.dma_start(out=w2t[:], in_=w2b[fc * 128 : (fc + 1) * 128, dc2 * 128 : (dc2 + 1) * 128])
                    for n2 in range(2):
                        nc.tensor.matmul(h2[n2][:], w2t[:], h1c[fc][:, HALVES[n2]],
                                         start=(fc == 0), stop=(fc == FC - 1))
                ot = tmp.tile([128, 1024], F32, tag="tmp")
                sa_rec = tmp.tile([128, 1024], F32, tag="tmp")
                for n2 in range(2):
                    nc.vector.tensor_mul(sa_rec[:, HALVES[n2]], sat[dc2][:, HALVES[n2]], rmsyB[n2][:])
                    nc.vector.tensor_add(ot[:, HALVES[n2]], sa_rec[:, HALVES[n2]], h2[n2][:])
                nc.sync.dma_start(out=outT[dc2 * 128 : (dc2 + 1) * 128, :], in_=ot[:])

    nc.compile()
    return nc


_NC = None


def _get_nc():
    global _NC
    if _NC is None:
        _NC = build()
    return _NC


def _build_in_maps(inputs):
    return _prep(**inputs)


def kernel(**inputs):
    in_maps = _prep(**inputs)
    res = run_bass_kernel_spmd(_get_nc(), in_maps, list(range(8)))
    out = np.empty((B, S, D), np.float32)
    for c in range(8):
        b, qh = c // 2, c % 2
        out[b, qh * SQ : (qh + 1) * SQ, :] = res.results[c]["outT"].T
    return out


def _prep(src, src_padding_mask, Wq, Wk, Wv, Wo, g1, g2, W1, V1, W2, **_):
    src = np.asarray(src, np.float32)
    valid = (~np.asarray(src_padding_mask, bool)).astype(np.float32)
    g1 = np.asarray(g1, np.float32)
    g2 = np.asarray(g2, np.float32)
    wq_cat = (np.transpose(np.asarray(Wq, np.float32), (1, 0, 2)).reshape(D, D)
              * g1[:, None]).astype(np.float32)
    wk_cat = (np.transpose(np.asarray(Wk, np.float32), (1, 0, 2)).reshape(D, D)
              * g1[:, None]).astype(np.float32)
    wv_cat = (np.transpose(np.asarray(Wv, np.float32), (1, 0, 2)).reshape(D, D)
              * g1[:, None]).astype(np.float32)
    wo_a = np.ascontiguousarray(np.asarray(Wo, np.float32))
    w1_s = np.ascontiguousarray(np.asarray(W1, np.float32) * g2[:, None])
    v1_s = np.ascontiguousarray(np.asarray(V1, np.float32) * g2[:, None])
    w2_b = np.asarray(W2, np.float32).astype(ml_dtypes.bfloat16)

    in_maps = []
    for c in range(8):
        b, qh = c // 2, c % 2
        roll = qh * SQ
        src_r = np.roll(src[b], -roll, axis=0)          # [S, D]
        srcT_c = np.ascontiguousarray(src_r.T)          # [D, S]
        km_c = np.ascontiguousarray(np.roll(valid[b], -roll)).reshape(S, 1)
        # additive softmax-denominator bias: +1e30 on padded query rows so
        # 1/denom ~ 0 there (reference zeroes those attention rows)
        qm_c = np.ascontiguousarray(((1.0 - km_c[0:SQ]) * 1e30).reshape(1, SQ))
        in_maps.append({
            "srcT": srcT_c, "kmask": km_c, "qmask": qm_c,
            "wq": wq_cat, "wk": wk_cat, "wv": wv_cat, "wo": wo_a,
            "w1": w1_s, "v1": v1_s, "w2b": w2_b,
        })
    return in_maps


# revision 34
# speedup vs baseline: 1.5390x; 1.0701x over previous
"""Trainium2 Bass kernel for a padded-attention transformer encoder layer.

Shapes (hardcoded): src [4, 2048, 1024], 16 heads x 64, d_ff 4096, 8 cores.

Sharding: each core computes the full layer for 1024 output tokens
(batch = core//2, token half = core%2). Inputs are host-transposed
(feature-major) and host-rolled so every core's query tokens are columns
0:1024 of its srcT; attention over keys is permutation invariant so the
roll only permutes the contraction order.

On-core dataflow (feature-on-partitions, fp32r matmuls, PSUM in 1-bank
[128,512] slots for deep PE run-ahead):
  xT = rmsnorm_T(srcT)                    (partition-dim reduce via ones-matmul)
  qT2/kT2 per head pair from xT; v (token-major, bf16) via xT-stationary matmuls
  scoresT[t,s] = kT.T @ qT (fp32r); expT = exp(scores/8) in bf16
  v_aug = [v*kmask | kmask] -> attnV (bf16) yields o and softmax denom at once
  o normalized by 1/(denom + 1e30*qpad), out-proj accumulated into saT
  yT = rmsnorm_T(saT) in place; h1 = silu(yT@W1)*(yT@V1) bf16 over dead xT space
  outT = yT*rms + h1 @ W2  (bf16 matmul, fp32 accum)
"""

import sys

sys.path.insert(0, "/opt/trn_rl_repo")

import numpy as np
import ml_dtypes

import concourse.bass as bass
import concourse.mybir as mybir
import concourse.tile as tile
from concourse import bacc
from concourse.bass_utils import run_bass_kernel_spmd

F32 = mybir.dt.float32
F32R = mybir.dt.float32r
BF16 = mybir.dt.bfloat16
EXP = mybir.ActivationFunctionType.Exp
SILU = mybir.ActivationFunctionType.Silu
SQRT = mybir.ActivationFunctionType.Sqrt

B, S, D, H, DK, DFF = 4, 2048, 1024, 16, 64, 4096
SQ = 1024          # query tokens per core
DC = D // 128      # 8 d-chunks
TC = S // 128      # 16 token chunks
FC = DFF // 128    # 32 dff chunks
NPAIR = H // 2     # 8 head pairs
SCALE = DK ** -0.5
H0, H1 = slice(0, 512), slice(512, 1024)
HALVES = (H0, H1)


def r32(ap):
    return ap.bitcast(F32R)


def build():
    nc = bacc.Bacc("TRN2", target_bir_lowering=False, debug=False, num_devices=8)

    srcT = nc.dram_tensor("srcT", [D, S], F32, kind="ExternalInput").ap()
    kmask = nc.dram_tensor("kmask", [S, 1], F32, kind="ExternalInput").ap()
    qmaskd = nc.dram_tensor("qmask", [1, SQ], F32, kind="ExternalInput").ap()
    wq = nc.dram_tensor("wq", [D, D], F32, kind="ExternalInput").ap()
    wk = nc.dram_tensor("wk", [D, D], F32, kind="ExternalInput").ap()
    wv = nc.dram_tensor("wv", [D, D], F32, kind="ExternalInput").ap()
    wo = nc.dram_tensor("wo", [D, D], F32, kind="ExternalInput").ap()
    w1 = nc.dram_tensor("w1", [D, DFF], F32, kind="ExternalInput").ap()
    v1 = nc.dram_tensor("v1", [D, DFF], F32, kind="ExternalInput").ap()
    w2b = nc.dram_tensor("w2b", [DFF, D], BF16, kind="ExternalInput").ap()
    outT = nc.dram_tensor("outT", [D, SQ], F32, kind="ExternalOutput").ap()

    # persistent SBUF arrays. h1 (bf16, FFN intermediate) aliases xt's bytes:
    # xt is fully consumed before the first h1 write (guaranteed through the
    # tracked saT dependency chain) and the aliased tensors keep the fp32r-
    # and bf16-consumed memory locations distinct for the BIR verifier.
    xt, h1t = [], []
    for i in range(DC):
        xt.append(nc.alloc_sbuf_tensor(f"xt{i}", [128, S], F32R).ap())
        off = nc.sbuf_base - S * 4
        h1t.append(nc.alloc_sbuf_tensor_at(f"h1t{i}", [128, 2 * S], BF16, offset=off).ap())
    # saT: residual stream, scaled in place to yT; residual restored as yT*rms
    sat = [nc.alloc_sbuf_tensor(f"sat{i}", [128, SQ], F32R).ap() for i in range(DC)]
    # v for one half-round (4 pairs = 8 heads), bf16, kmask-augmented col 64
    vq = [nc.alloc_sbuf_tensor(f"vq{i}", [128, 8 * 65], BF16).ap() for i in range(TC)]

    with nc.allow_low_precision(reason="fp32r/bf16 matmul operand rounding; fp32 PSUM accumulation"), \
         tile.TileContext(nc) as tc:
        with (
            tc.tile_pool(name="kt2p", bufs=2) as kt2p,
            tc.tile_pool(name="qzp", bufs=2) as qzp,
            tc.tile_pool(name="expp", bufs=6) as expp,
            tc.tile_pool(name="otnp", bufs=2) as otnp,
            tc.tile_pool(name="tmp", bufs=3) as tmp,
            tc.tile_pool(name="wst", bufs=8) as wst,
            tc.tile_pool(name="w2st", bufs=4) as w2st,
            tc.tile_pool(name="wost", bufs=2) as wost,
            tc.tile_pool(name="consts", bufs=1) as consts,
            tc.tile_pool(name="sm", bufs=2) as sm,
            tc.tile_pool(name="ps", bufs=8, space="PSUM") as ps,
        ):
            # ---- constants ----
            onesf = consts.tile([128, 128], F32, tag="onesf")
            nc.vector.memset(onesf[:], 1.0)
            ones4 = consts.tile([128, 4], F32R, tag="ones4")
            nc.vector.tensor_copy(ones4[:], onesf[:, 0:4])
            ones1 = consts.tile([1, 128], F32R, tag="ones1")
            nc.vector.tensor_copy(ones1[:], onesf[0:1, :])
            km = consts.tile([128, TC], F32, tag="km")
            for ti in range(TC):
                nc.sync.dma_start(out=km[:, ti : ti + 1], in_=kmask[ti * 128 : (ti + 1) * 128, :])
            qm = consts.tile([1, SQ], F32, tag="qm")
            nc.sync.dma_start(out=qm[:], in_=qmaskd[:])

            def vround_weights(vr):
                wvts = []
                for dc in range(DC):
                    wvt = wst.tile([128, 512], F32R, tag="wst", name="wvt")
                    nc.sync.dma_start(
                        out=wvt[:],
                        in_=r32(wv[dc * 128 : (dc + 1) * 128, vr * 512 : (vr + 1) * 512]),
                    )
                    wvts.append(wvt)
                return wvts

            def emit_vround(wvts, tis):
                for ti in tis:
                    vps = ps.tile([128, 512], F32, tag="ps", name="vps")
                    for dc in range(DC):
                        nc.tensor.matmul(
                            vps[:],
                            xt[dc][:, ti * 128 : (ti + 1) * 128],
                            wvts[dc][:],
                            start=(dc == 0), stop=(dc == DC - 1),
                        )
                    # vq[ti] = [v*km | km] per head: [128, 8, 65] bf16
                    dst = vq[ti].rearrange("p (h c) -> p h c", c=65)
                    src3 = vps[:].rearrange("p (h c) -> p h c", c=64)
                    nc.vector.tensor_scalar_mul(dst[:, :, 0:64], src3, km[:, ti : ti + 1])
                    for l in range(8):
                        nc.vector.tensor_copy(dst[:, l, 64:65], km[:, ti : ti + 1])

            # ---- P0: xT = rmsnorm_T(srcT), in two 1024-col halves; the first
            # v half-round runs on each xT half as it completes to keep PE busy
            wvts0 = vround_weights(0)
            for th in range(2):
                hs = slice(th * 1024, (th + 1) * 1024)
                ssq = [ps.tile([4, 512], F32, tag="ps", name=f"ssq{n2}") for n2 in range(2)]
                for dc in range(DC):
                    ld = tmp.tile([128, 1024], F32R, tag="tmp")
                    nc.sync.dma_start(out=ld[:], in_=r32(srcT[dc * 128 : (dc + 1) * 128, hs]))
                    sq = tmp.tile([128, 1024], F32R, tag="tmp")
                    nc.vector.tensor_mul(sq[:], ld[:], ld[:])
                    for n2 in range(2):
                        nc.tensor.matmul(ssq[n2][:], ones4[:], sq[:, HALVES[n2]],
                                         start=(dc == 0), stop=(dc == DC - 1))
                rms = sm.tile([1, 1024], F32R, tag="rsq", name="rms")
                for n2 in range(2):
                    nc.scalar.activation(rms[:, HALVES[n2]], ssq[n2][0:1, :], SQRT, scale=1.0 / D)
                rmsB = [ps.tile([128, 512], F32, tag="ps", name=f"rmsB{n2}") for n2 in range(2)]
                for n2 in range(2):
                    nc.tensor.matmul(rmsB[n2][:], ones1[:], rms[:, HALVES[n2]], start=True, stop=True)
                invB = tmp.tile([128, 1024], F32, tag="tmp")
                for n2 in range(2):
                    nc.vector.reciprocal_approx_fast(out=invB[:, HALVES[n2]], in_=rmsB[n2][:])
                for dc in range(DC):
                    ld2 = tmp.tile([128, 1024], F32R, tag="tmp")
                    nc.sync.dma_start(out=ld2[:], in_=r32(srcT[dc * 128 : (dc + 1) * 128, hs]))
                    nc.vector.tensor_mul(xt[dc][:, hs], ld2[:], invB[:])
                emit_vround(wvts0, range(th * 8, (th + 1) * 8))

            # ---- P1: attention ----
            # out-proj of pair p is emitted after pair p+1's kT2/qT2 matmuls so
            # the PE never stalls on the softmax-denominator reciprocal chain.
            def emit_outproj(p, otn):
                for dc2 in range(DC):
                    wot = wost.tile([128, 128], F32R, tag="wost", name="wot")
                    nc.sync.dma_start(out=wot[:], in_=r32(wo[p * 128 : (p + 1) * 128, dc2 * 128 : (dc2 + 1) * 128]))
                    for n2 in range(2):
                        pp = ps.tile([128, 512], F32, tag="ps", name="pp")
                        nc.tensor.matmul(pp[:], wot[:], otn[:, HALVES[n2]], start=True, stop=True)
                        if p == 0:
                            srcq = tmp.tile([128, 512], F32, tag="tmp", name="srcq")
                            nc.sync.dma_start(out=srcq[:], in_=srcT[dc2 * 128 : (dc2 + 1) * 128, th_sq(n2)])
                            nc.vector.tensor_add(sat[dc2][:, HALVES[n2]], pp[:], srcq[:])
                        else:
                            nc.vector.tensor_add(sat[dc2][:, HALVES[n2]], sat[dc2][:, HALVES[n2]], pp[:])

            def th_sq(n2):
                return slice(n2 * 512, (n2 + 1) * 512)

            pending = None  # (pair index, otn tile) awaiting out-proj
            for p in range(NPAIR):
                vr, lp = p // 4, p % 4
                if p == 4:
                    wvts1 = vround_weights(1)
                    emit_vround(wvts1, range(TC))

                # kT2 / qT2 for this pair
                kps = [ps.tile([128, 512], F32, tag="ps", name=f"kps{j}") for j in range(4)]
                for dc in range(DC):
                    wkt = wst.tile([128, 128], F32R, tag="wst", name="wkt")
                    nc.sync.dma_start(out=wkt[:], in_=r32(wk[dc * 128 : (dc + 1) * 128, p * 128 : (p + 1) * 128]))
                    for j in range(4):
                        nc.tensor.matmul(kps[j][:], wkt[:], xt[dc][:, j * 512 : (j + 1) * 512],
                                         start=(dc == 0), stop=(dc == DC - 1))
                kt2 = kt2p.tile([128, S], F32R, tag="kt2")
                for j in range(4):
                    nc.vector.tensor_copy(kt2[:, j * 512 : (j + 1) * 512], kps[j][:])
                qps = [ps.tile([128, 512], F32, tag="ps", name=f"qps{j}") for j in range(2)]
                for dc in range(DC):
                    wqt = wst.tile([128, 128], F32R, tag="wst", name="wqt")
                    nc.sync.dma_start(out=wqt[:], in_=r32(wq[dc * 128 : (dc + 1) * 128, p * 128 : (p + 1) * 128]))
                    for j in range(2):
                        nc.tensor.matmul(qps[j][:], wqt[:], xt[dc][:, j * 512 : (j + 1) * 512],
                                         start=(dc == 0), stop=(dc == DC - 1))
                # zero-padded per-head q: scores contract at K=128 (full array)
                qzA = qzp.tile([128, SQ], F32R, tag="qzA")
                qzB = qzp.tile([128, SQ], F32R, tag="qzB")
                for j in range(2):
                    js = slice(j * 512, (j + 1) * 512)
                    nc.vector.tensor_copy(qzA[0:64, js], qps[j][0:64, :])
                    nc.vector.tensor_scalar_mul(qzA[64:128, js], qps[j][64:128, :], 0.0)
                    nc.vector.tensor_copy(qzB[64:128, js], qps[j][64:128, :])
                    nc.vector.tensor_scalar_mul(qzB[0:64, js], qps[j][0:64, :], 0.0)

                if pending is not None:
                    emit_outproj(*pending)
                otn = otnp.tile([128, 1024], F32R, tag="otn")
                for h in range(2):
                    l = (p % 4) * 2 + h  # head slot within current vq half-round
                    oT = [ps.tile([128, 512], F32, tag="ps", name=f"oT{n2}") for n2 in range(2)]
                    prev_ex = None
                    for ti in range(TC):
                        exs = []
                        for n2 in range(2):
                            sc = ps.tile([128, 512], F32, tag="ps", name="sc")
                            qz = qzA if h == 0 else qzB
                            nc.tensor.matmul(
                                sc[:],
                                kt2[:, ti * 128 : (ti + 1) * 128],
                                qz[:, HALVES[n2]],
                                start=True, stop=True,
                            )
                            ex = expp.tile([128, 512], BF16, tag="exp", name="ex")
                            nc.scalar.activation(ex[:], sc[:], EXP, scale=SCALE)
                            exs.append(ex)
                        if prev_ex is not None:
                            pti, pex = prev_ex
                            for n2 in range(2):
                                nc.tensor.matmul(
                                    oT[n2][0:65, :],
                                    vq[pti][:, l * 65 : (l + 1) * 65],
                                    pex[n2][:],
                                    start=(pti == 0), stop=False,
                                )
                        prev_ex = (ti, exs)
                    pti, pex = prev_ex
                    for n2 in range(2):
                        nc.tensor.matmul(
                            oT[n2][0:65, :],
                            vq[pti][:, l * 65 : (l + 1) * 65],
                            pex[n2][:],
                            start=False, stop=True,
                        )
                    # normalize rows: o/denom, denom += 1e30 on masked queries
                    rsq = sm.tile([1, 1024], F32R, tag="rsq")
                    for n2 in range(2):
                        nc.vector.tensor_add(rsq[:, HALVES[n2]], oT[n2][64:65, :], qm[:, HALVES[n2]])
                    recB = tmp.tile([128, 1024], F32, tag="tmp")
                    for n2 in range(2):
                        rB = ps.tile([128, 512], F32, tag="ps", name="rB")
                        nc.tensor.matmul(rB[0:64, :], ones1[:, 0:64], rsq[:, HALVES[n2]],
                                         start=True, stop=True)
                        nc.vector.reciprocal_approx_fast(out=recB[0:64, HALVES[n2]], in_=rB[0:64, :])
                    for n2 in range(2):
                        nc.vector.tensor_mul(otn[h * 64 : (h + 1) * 64, HALVES[n2]],
                                             oT[n2][0:64, :], recB[0:64, HALVES[n2]])
                pending = (p, otn)
            emit_outproj(*pending)

            # ---- P2: saT -> yT in place; keep rmsy for the final residual ----
            ssy = [ps.tile([4, 512], F32, tag="ps", name=f"ssy{n2}") for n2 in range(2)]
            for dc in range(DC):
                sqy = tmp.tile([128, 1024], F32R, tag="tmp")
                nc.vector.tensor_mul(sqy[:], sat[dc][:], sat[dc][:])
                for n2 in range(2):
                    nc.tensor.matmul(ssy[n2][:], ones4[:], sqy[:, HALVES[n2]],
                                     start=(dc == 0), stop=(dc == DC - 1))
            rmsy = consts.tile([1, 1024], F32R, tag="rmsy")
            for n2 in range(2):
                nc.scalar.activation(rmsy[:, HALVES[n2]], ssy[n2][0:1, :], SQRT, scale=1.0 / D)
            invyB = tmp.tile([128, 1024], F32, tag="tmp")
            for n2 in range(2):
                rmsyB2 = ps.tile([128, 512], F32, tag="ps", name="rmsyB2")
                nc.tensor.matmul(rmsyB2[:], ones1[:], rmsy[:, HALVES[n2]], start=True, stop=True)
                nc.vector.reciprocal_approx_fast(out=invyB[:, HALVES[n2]], in_=rmsyB2[:])
            for dc in range(DC):
                nc.vector.tensor_mul(sat[dc][:], sat[dc][:], invyB[:])

            # ---- P3: h1 = silu(yT@W1)*(yT@V1) -> bf16, overlaid on xt space ----
            h1c = []
            for fc in range(FC):
                h1c.append(h1t[fc // 4][:, (fc % 4) * 1024 : (fc % 4 + 1) * 1024])
            for fc in range(FC):
                h1w = [ps.tile([128, 512], F32, tag="ps", name=f"h1w{n2}") for n2 in range(2)]
                h1v = [ps.tile([128, 512], F32, tag="ps", name=f"h1v{n2}") for n2 in range(2)]
                for dc in range(DC):
                    w1t = wst.tile([128, 128], F32R, tag="wst", name="w1t")
                    nc.sync.dma_start(out=w1t[:], in_=r32(w1[dc * 128 : (dc + 1) * 128, fc * 128 : (fc + 1) * 128]))
                    v1t = wst.tile([128, 128], F32R, tag="wst", name="v1t")
                    nc.scalar.dma_start(out=v1t[:], in_=r32(v1[dc * 128 : (dc + 1) * 128, fc * 128 : (fc + 1) * 128]))
                    for n2 in range(2):
                        nc.tensor.matmul(h1w[n2][:], w1t[:], sat[dc][:, HALVES[n2]],
                                         start=(dc == 0), stop=(dc == DC - 1))
                        nc.tensor.matmul(h1v[n2][:], v1t[:], sat[dc][:, HALVES[n2]],
                                         start=(dc == 0), stop=(dc == DC - 1))
                sil = tmp.tile([128, 1024], F32, tag="tmp")
                for n2 in range(2):
                    nc.scalar.activation(sil[:, HALVES[n2]], h1w[n2][:], SILU)
                    nc.vector.tensor_mul(h1c[fc][:, HALVES[n2]], sil[:, HALVES[n2]], h1v[n2][:])

            # ---- P4: outT = yT*rmsy + h1 @ W2   (saT reconstructed) ----
            rmsyB = [ps.tile([128, 512], F32, tag="ps", name=f"rmsyB{n2}") for n2 in range(2)]
            for n2 in range(2):
                nc.tensor.matmul(rmsyB[n2][:], ones1[:], rmsy[:, HALVES[n2]], start=True, stop=True)
            for dg in range(DC // 2):
                h2 = [ps.tile([128, 512], F32, tag="ps", name=f"h2{j}") for j in range(4)]
                for fc in range(FC):
                    w2t = w2st.tile([128, 256], BF16, tag="w2st")
                    nc.scalar.dma_start(out=w2t[:], in_=w2b[fc * 128 : (fc + 1) * 128, dg * 256 : (dg + 1) * 256])
                    for d2 in range(2):
                        for n2 in range(2):
                            nc.tensor.matmul(h2[d2 * 2 + n2][:], w2t[:, d2 * 128 : (d2 + 1) * 128],
                                             h1c[fc][:, HALVES[n2]],
                                             start=(fc == 0), stop=(fc == FC - 1))
                for d2 in range(2):
                    dc2 = dg * 2 + d2
                    ot = tmp.tile([128, 1024], F32, tag="tmp", name="ot")
                    sa_rec = tmp.tile([128, 1024], F32, tag="tmp", name="sa_rec")
                    for n2 in range(2):
                        nc.vector.tensor_mul(sa_rec[:, HALVES[n2]], sat[dc2][:, HALVES[n2]], rmsyB[n2][:])
                        nc.vector.tensor_add(ot[:, HALVES[n2]], sa_rec[:, HALVES[n2]], h2[d2 * 2 + n2][:])
                    nc.sync.dma_start(out=outT[dc2 * 128 : (dc2 + 1) * 128, :], in_=ot[:])

    nc.compile()
    return nc


_NC = None


def _get_nc():
    global _NC
    if _NC is None:
        _NC = build()
    return _NC


def _build_in_maps(inputs):
    return _prep(**inputs)


def kernel(**inputs):
    in_maps = _prep(**inputs)
    res = run_bass_kernel_spmd(_get_nc(), in_maps, list(range(8)))
    out = np.empty((B, S, D), np.float32)
    for c in range(8):
        b, qh = c // 2, c % 2
        out[b, qh * SQ : (qh + 1) * SQ, :] = res.results[c]["outT"].T
    return out


def _prep(src, src_padding_mask, Wq, Wk, Wv, Wo, g1, g2, W1, V1, W2, **_):
    src = np.asarray(src, np.float32)
    valid = (~np.asarray(src_padding_mask, bool)).astype(np.float32)
    g1 = np.asarray(g1, np.float32)
    g2 = np.asarray(g2, np.float32)
    wq_cat = (np.transpose(np.asarray(Wq, np.float32), (1, 0, 2)).reshape(D, D)
              * g1[:, None]).astype(np.float32)
    wk_cat = (np.transpose(np.asarray(Wk, np.float32), (1, 0, 2)).reshape(D, D)
              * g1[:, None]).astype(np.float32)
    wv_cat = (np.transpose(np.asarray(Wv, np.float32), (1, 0, 2)).reshape(D, D)
              * g1[:, None]).astype(np.float32)
    wo_a = np.ascontiguousarray(np.asarray(Wo, np.float32))
    w1_s = np.ascontiguousarray(np.asarray(W1, np.float32) * g2[:, None])
    v1_s = np.ascontiguousarray(np.asarray(V1, np.float32) * g2[:, None])
    w2_b = np.asarray(W2, np.float32).astype(ml_dtypes.bfloat16)

    in_maps = []
    for c in range(8):
        b, qh = c // 2, c % 2
        roll = qh * SQ
        src_r = np.roll(src[b], -roll, axis=0)          # [S, D]
        srcT_c = np.ascontiguousarray(src_r.T)          # [D, S]
        km_c = np.ascontiguousarray(np.roll(valid[b], -roll)).reshape(S, 1)
        # additive softmax-denominator bias: +1e30 on padded query rows so
        # 1/denom ~ 0 there (reference zeroes those attention rows)
        qm_c = np.ascontiguousarray(((1.0 - km_c[0:SQ]) * 1e30).reshape(1, SQ))
        in_maps.append({
            "srcT": srcT_c, "kmask": km_c, "qmask": qm_c,
            "wq": wq_cat, "wk": wk_cat, "wv": wv_cat, "wo": wo_a,
            "w1": w1_s, "v1": v1_s, "w2b": w2_b,
        })
    return in_maps


# revision 35
# speedup vs baseline: 1.5467x; 1.0050x over previous
"""Trainium2 Bass kernel for a padded-attention transformer encoder layer.

Shapes (hardcoded): src [4, 2048, 1024], 16 heads x 64, d_ff 4096, 8 cores.

Sharding: each core computes the full layer for 1024 output tokens
(batch = core//2, token half = core%2). Inputs are host-transposed
(feature-major) and host-rolled so every core's query tokens are columns
0:1024 of its srcT; attention over keys is permutation invariant so the
roll only permutes the contraction order.

On-core dataflow (feature-on-partitions, fp32r matmuls, PSUM in 1-bank
[128,512] slots for deep PE run-ahead):
  xT = rmsnorm_T(srcT)                    (partition-dim reduce via ones-matmul)
  qT2/kT2 per head pair from xT; v (token-major, bf16) via xT-stationary matmuls
  scoresT[t,s] = kT.T @ qT (fp32r); expT = exp(scores/8) in bf16
  v_aug = [v*kmask | kmask] -> attnV (bf16) yields o and softmax denom at once
  o normalized by 1/(denom + 1e30*qpad), out-proj accumulated into saT
  yT = rmsnorm_T(saT) in place; h1 = silu(yT@W1)*(yT@V1) bf16 over dead xT space
  outT = yT*rms + h1 @ W2  (bf16 matmul, fp32 accum)
"""

import sys

sys.path.insert(0, "/opt/trn_rl_repo")

import numpy as np
import ml_dtypes

import concourse.bass as bass
import concourse.mybir as mybir
import concourse.tile as tile
from concourse import bacc
from concourse.bass_utils import run_bass_kernel_spmd

F32 = mybir.dt.float32
F32R = mybir.dt.float32r
BF16 = mybir.dt.bfloat16
EXP = mybir.ActivationFunctionType.Exp
SILU = mybir.ActivationFunctionType.Silu
SQRT = mybir.ActivationFunctionType.Sqrt

B, S, D, H, DK, DFF = 4, 2048, 1024, 16, 64, 4096
SQ = 1024          # query tokens per core
DC = D // 128      # 8 d-chunks
TC = S // 128      # 16 token chunks
FC = DFF // 128    # 32 dff chunks
NPAIR = H // 2     # 8 head pairs
SCALE = DK ** -0.5
H0, H1 = slice(0, 512), slice(512, 1024)
HALVES = (H0, H1)


def r32(ap):
    return ap.bitcast(F32R)


def build():
    nc = bacc.Bacc("TRN2", target_bir_lowering=False, debug=False, num_devices=8)

    srcT = nc.dram_tensor("srcT", [D, S], F32, kind="ExternalInput").ap()
    kmask = nc.dram_tensor("kmask", [S, 1], F32, kind="ExternalInput").ap()
    qmaskd = nc.dram_tensor("qmask", [1, SQ], F32, kind="ExternalInput").ap()
    wq = nc.dram_tensor("wq", [D, D], F32, kind="ExternalInput").ap()
    wk = nc.dram_tensor("wk", [D, D], F32, kind="ExternalInput").ap()
    wv = nc.dram_tensor("wv", [D, D], F32, kind="ExternalInput").ap()
    wo = nc.dram_tensor("wo", [D, D], F32, kind="ExternalInput").ap()
    w1 = nc.dram_tensor("w1", [D, DFF], F32, kind="ExternalInput").ap()
    v1 = nc.dram_tensor("v1", [D, DFF], F32, kind="ExternalInput").ap()
    w2b = nc.dram_tensor("w2b", [DFF, D], BF16, kind="ExternalInput").ap()
    outT = nc.dram_tensor("outT", [D, SQ], F32, kind="ExternalOutput").ap()

    # persistent SBUF arrays. h1 (bf16, FFN intermediate) aliases xt's bytes:
    # xt is fully consumed before the first h1 write (guaranteed through the
    # tracked saT dependency chain) and the aliased tensors keep the fp32r-
    # and bf16-consumed memory locations distinct for the BIR verifier.
    xt, h1t = [], []
    for i in range(DC):
        xt.append(nc.alloc_sbuf_tensor(f"xt{i}", [128, S], F32R).ap())
        off = nc.sbuf_base - S * 4
        h1t.append(nc.alloc_sbuf_tensor_at(f"h1t{i}", [128, 2 * S], BF16, offset=off).ap())
    # saT: residual stream, scaled in place to yT; residual restored as yT*rms
    sat = [nc.alloc_sbuf_tensor(f"sat{i}", [128, SQ], F32R).ap() for i in range(DC)]
    # v for one half-round (4 pairs = 8 heads), bf16, kmask-augmented col 64
    vq = [nc.alloc_sbuf_tensor(f"vq{i}", [128, 8 * 65], BF16).ap() for i in range(TC)]
    # zero-padded per-head q (scores contract at K=128); zero halves written once
    qzA = nc.alloc_sbuf_tensor("qzA", [128, SQ], F32R).ap()
    qzB = nc.alloc_sbuf_tensor("qzB", [128, SQ], F32R).ap()

    with nc.allow_low_precision(reason="fp32r/bf16 matmul operand rounding; fp32 PSUM accumulation"), \
         tile.TileContext(nc) as tc:
        with (
            tc.tile_pool(name="kt2p", bufs=2) as kt2p,
            tc.tile_pool(name="expp", bufs=6) as expp,
            tc.tile_pool(name="otnp", bufs=2) as otnp,
            tc.tile_pool(name="tmp", bufs=3) as tmp,
            tc.tile_pool(name="wst", bufs=8) as wst,
            tc.tile_pool(name="w2st", bufs=4) as w2st,
            tc.tile_pool(name="wost", bufs=2) as wost,
            tc.tile_pool(name="consts", bufs=1) as consts,
            tc.tile_pool(name="sm", bufs=2) as sm,
            tc.tile_pool(name="ps", bufs=8, space="PSUM") as ps,
        ):
            # ---- constants ----
            onesf = consts.tile([128, 128], F32, tag="onesf")
            nc.vector.memset(onesf[:], 1.0)
            ones4 = consts.tile([128, 4], F32R, tag="ones4")
            nc.vector.tensor_copy(ones4[:], onesf[:, 0:4])
            ones1 = consts.tile([1, 128], F32R, tag="ones1")
            nc.vector.tensor_copy(ones1[:], onesf[0:1, :])
            km = consts.tile([128, TC], F32, tag="km")
            for ti in range(TC):
                nc.sync.dma_start(out=km[:, ti : ti + 1], in_=kmask[ti * 128 : (ti + 1) * 128, :])
            qm = consts.tile([1, SQ], F32, tag="qm")
            nc.sync.dma_start(out=qm[:], in_=qmaskd[:])
            zt = tmp.tile([128, 1024], F32, tag="tmp", name="zt")
            nc.vector.memset(zt[:], 0.0)
            nc.vector.tensor_copy(qzA[64:128, :], zt[64:128, :])
            nc.vector.tensor_copy(qzB[0:64, :], zt[0:64, :])

            def vround_weights(vr):
                wvts = []
                for dc in range(DC):
                    wvt = wst.tile([128, 512], F32R, tag="wst", name="wvt")
                    nc.sync.dma_start(
                        out=wvt[:],
                        in_=r32(wv[dc * 128 : (dc + 1) * 128, vr * 512 : (vr + 1) * 512]),
                    )
                    wvts.append(wvt)
                return wvts

            def emit_vround(wvts, tis):
                for ti in tis:
                    vps = ps.tile([128, 512], F32, tag="ps", name="vps")
                    for dc in range(DC):
                        nc.tensor.matmul(
                            vps[:],
                            xt[dc][:, ti * 128 : (ti + 1) * 128],
                            wvts[dc][:],
                            start=(dc == 0), stop=(dc == DC - 1),
                        )
                    # vq[ti] = [v*km | km] per head: [128, 8, 65] bf16
                    dst = vq[ti].rearrange("p (h c) -> p h c", c=65)
                    src3 = vps[:].rearrange("p (h c) -> p h c", c=64)
                    nc.vector.tensor_scalar_mul(dst[:, :, 0:64], src3, km[:, ti : ti + 1])
                    nc.vector.tensor_scalar(
                        out=dst[:, :, 64:65], in0=dst[:, :, 0:1],
                        scalar1=0.0, scalar2=km[:, ti : ti + 1],
                        op0=mybir.AluOpType.mult, op1=mybir.AluOpType.add)

            # ---- P0: xT = rmsnorm_T(srcT), in two 1024-col halves; the first
            # v half-round runs on each xT half as it completes to keep PE busy
            wvts0 = vround_weights(0)
            for th in range(2):
                hs = slice(th * 1024, (th + 1) * 1024)
                ssq = [ps.tile([4, 512], F32, tag="ps", name=f"ssq{n2}") for n2 in range(2)]
                for dc in range(DC):
                    ld = tmp.tile([128, 1024], F32R, tag="tmp")
                    nc.sync.dma_start(out=ld[:], in_=r32(srcT[dc * 128 : (dc + 1) * 128, hs]))
                    sq = tmp.tile([128, 1024], F32R, tag="tmp")
                    nc.scalar.activation(sq[:], ld[:], mybir.ActivationFunctionType.Square)
                    for n2 in range(2):
                        nc.tensor.matmul(ssq[n2][:], ones4[:], sq[:, HALVES[n2]],
                                         start=(dc == 0), stop=(dc == DC - 1))
                rms = sm.tile([1, 1024], F32R, tag="rsq", name="rms")
                for n2 in range(2):
                    nc.scalar.activation(rms[:, HALVES[n2]], ssq[n2][0:1, :], SQRT, scale=1.0 / D)
                rmsB = [ps.tile([128, 512], F32, tag="ps", name=f"rmsB{n2}") for n2 in range(2)]
                for n2 in range(2):
                    nc.tensor.matmul(rmsB[n2][:], ones1[:], rms[:, HALVES[n2]], start=True, stop=True)
                invB = tmp.tile([128, 1024], F32, tag="tmp")
                for n2 in range(2):
                    nc.vector.reciprocal_approx_fast(out=invB[:, HALVES[n2]], in_=rmsB[n2][:])
                for dc in range(DC):
                    ld2 = tmp.tile([128, 1024], F32R, tag="tmp")
                    nc.sync.dma_start(out=ld2[:], in_=r32(srcT[dc * 128 : (dc + 1) * 128, hs]))
                    nc.vector.tensor_mul(xt[dc][:, hs], ld2[:], invB[:])
                emit_vround(wvts0, range(th * 8, (th + 1) * 8))

            # ---- P1: attention ----
            # out-proj of pair p is emitted after pair p+1's kT2/qT2 matmuls so
            # the PE never stalls on the softmax-denominator reciprocal chain.
            def emit_outproj(p, otn):
                for dc2 in range(DC):
                    wot = wost.tile([128, 128], F32R, tag="wost", name="wot")
                    nc.sync.dma_start(out=wot[:], in_=r32(wo[p * 128 : (p + 1) * 128, dc2 * 128 : (dc2 + 1) * 128]))
                    for n2 in range(2):
                        pp = ps.tile([128, 512], F32, tag="ps", name="pp")
                        nc.tensor.matmul(pp[:], wot[:], otn[:, HALVES[n2]], start=True, stop=True)
                        if p == 0:
                            srcq = tmp.tile([128, 512], F32, tag="tmp", name="srcq")
                            nc.sync.dma_start(out=srcq[:], in_=srcT[dc2 * 128 : (dc2 + 1) * 128, th_sq(n2)])
                            nc.vector.tensor_add(sat[dc2][:, HALVES[n2]], pp[:], srcq[:])
                        else:
                            nc.vector.tensor_add(sat[dc2][:, HALVES[n2]], sat[dc2][:, HALVES[n2]], pp[:])

            def th_sq(n2):
                return slice(n2 * 512, (n2 + 1) * 512)

            pending = None  # (pair index, otn tile) awaiting out-proj
            for p in range(NPAIR):
                vr, lp = p // 4, p % 4
                if p == 4:
                    wvts1 = vround_weights(1)
                    emit_vround(wvts1, range(TC))

                # kT2 / qT2 for this pair
                kps = [ps.tile([128, 512], F32, tag="ps", name=f"kps{j}") for j in range(4)]
                for dc in range(DC):
                    wkt = wst.tile([128, 128], F32R, tag="wst", name="wkt")
                    nc.sync.dma_start(out=wkt[:], in_=r32(wk[dc * 128 : (dc + 1) * 128, p * 128 : (p + 1) * 128]))
                    for j in range(4):
                        nc.tensor.matmul(kps[j][:], wkt[:], xt[dc][:, j * 512 : (j + 1) * 512],
                                         start=(dc == 0), stop=(dc == DC - 1))
                kt2 = kt2p.tile([128, S], F32R, tag="kt2")
                for j in range(4):
                    if j % 2 == 0:
                        nc.vector.tensor_copy(kt2[:, j * 512 : (j + 1) * 512], kps[j][:])
                    else:
                        nc.scalar.copy(kt2[:, j * 512 : (j + 1) * 512], kps[j][:])
                qps = [ps.tile([128, 512], F32, tag="ps", name=f"qps{j}") for j in range(2)]
                for dc in range(DC):
                    wqt = wst.tile([128, 128], F32R, tag="wst", name="wqt")
                    nc.sync.dma_start(out=wqt[:], in_=r32(wq[dc * 128 : (dc + 1) * 128, p * 128 : (p + 1) * 128]))
                    for j in range(2):
                        nc.tensor.matmul(qps[j][:], wqt[:], xt[dc][:, j * 512 : (j + 1) * 512],
                                         start=(dc == 0), stop=(dc == DC - 1))
                for j in range(2):
                    js = slice(j * 512, (j + 1) * 512)
                    nc.vector.tensor_copy(qzA[0:64, js], qps[j][0:64, :])
                    nc.scalar.copy(qzB[64:128, js], qps[j][64:128, :])

                if pending is not None:
                    emit_outproj(*pending)
                otn = otnp.tile([128, 1024], F32R, tag="otn")
                for h in range(2):
                    l = (p % 4) * 2 + h  # head slot within current vq half-round
                    oT = [ps.tile([128, 512], F32, tag="ps", name=f"oT{n2}") for n2 in range(2)]
                    prev_ex = None
                    for ti in range(TC):
                        exs = []
                        for n2 in range(2):
                            sc = ps.tile([128, 512], F32, tag="ps", name="sc")
                            qz = qzA if h == 0 else qzB
                            nc.tensor.matmul(
                                sc[:],
                                kt2[:, ti * 128 : (ti + 1) * 128],
                                qz[:, HALVES[n2]],
                                start=True, stop=True,
                            )
                            ex = expp.tile([128, 512], BF16, tag="exp", name="ex")
                            nc.scalar.activation(ex[:], sc[:], EXP, scale=SCALE)
                            exs.append(ex)
                        if prev_ex is not None:
                            pti, pex = prev_ex
                            for n2 in range(2):
                                nc.tensor.matmul(
                                    oT[n2][0:65, :],
                                    vq[pti][:, l * 65 : (l + 1) * 65],
                                    pex[n2][:],
                                    start=(pti == 0), stop=False,
                                )
                        prev_ex = (ti, exs)
                    pti, pex = prev_ex
                    for n2 in range(2):
                        nc.tensor.matmul(
                            oT[n2][0:65, :],
                            vq[pti][:, l * 65 : (l + 1) * 65],
                            pex[n2][:],
                            start=False, stop=True,
                        )
                    # normalize rows: o/denom, denom += 1e30 on masked queries
                    rsq = sm.tile([1, 1024], F32R, tag="rsq")
                    for n2 in range(2):
                        nc.vector.tensor_add(rsq[:, HALVES[n2]], oT[n2][64:65, :], qm[:, HALVES[n2]])
                    recB = tmp.tile([128, 1024], F32, tag="tmp")
                    for n2 in range(2):
                        rB = ps.tile([128, 512], F32, tag="ps", name="rB")
                        nc.tensor.matmul(rB[0:64, :], ones1[:, 0:64], rsq[:, HALVES[n2]],
                                         start=True, stop=True)
                        nc.vector.reciprocal_approx_fast(out=recB[0:64, HALVES[n2]], in_=rB[0:64, :])
                    for n2 in range(2):
                        nc.vector.tensor_mul(otn[h * 64 : (h + 1) * 64, HALVES[n2]],
                                             oT[n2][0:64, :], recB[0:64, HALVES[n2]])
                pending = (p, otn)
            emit_outproj(*pending)

            # ---- P2: saT -> yT in place; keep rmsy for the final residual ----
            ssy = [ps.tile([4, 512], F32, tag="ps", name=f"ssy{n2}") for n2 in range(2)]
            for dc in range(DC):
                sqy = tmp.tile([128, 1024], F32R, tag="tmp")
                nc.vector.tensor_mul(sqy[:], sat[dc][:], sat[dc][:])
                for n2 in range(2):
                    nc.tensor.matmul(ssy[n2][:], ones4[:], sqy[:, HALVES[n2]],
                                     start=(dc == 0), stop=(dc == DC - 1))
            rmsy = consts.tile([1, 1024], F32R, tag="rmsy")
            for n2 in range(2):
                nc.scalar.activation(rmsy[:, HALVES[n2]], ssy[n2][0:1, :], SQRT, scale=1.0 / D)
            invyB = tmp.tile([128, 1024], F32, tag="tmp")
            for n2 in range(2):
                rmsyB2 = ps.tile([128, 512], F32, tag="ps", name="rmsyB2")
                nc.tensor.matmul(rmsyB2[:], ones1[:], rmsy[:, HALVES[n2]], start=True, stop=True)
                nc.vector.reciprocal_approx_fast(out=invyB[:, HALVES[n2]], in_=rmsyB2[:])
            for dc in range(DC):
                nc.vector.tensor_mul(sat[dc][:], sat[dc][:], invyB[:])

            # ---- P3: h1 = silu(yT@W1)*(yT@V1) -> bf16, overlaid on xt space ----
            h1c = []
            for fc in range(FC):
                h1c.append(h1t[fc // 4][:, (fc % 4) * 1024 : (fc % 4 + 1) * 1024])
            for fc in range(FC):
                h1w = [ps.tile([128, 512], F32, tag="ps", name=f"h1w{n2}") for n2 in range(2)]
                h1v = [ps.tile([128, 512], F32, tag="ps", name=f"h1v{n2}") for n2 in range(2)]
                for dc in range(DC):
                    w1t = wst.tile([128, 128], F32R, tag="wst", name="w1t")
                    nc.sync.dma_start(out=w1t[:], in_=r32(w1[dc * 128 : (dc + 1) * 128, fc * 128 : (fc + 1) * 128]))
                    v1t = wst.tile([128, 128], F32R, tag="wst", name="v1t")
                    nc.scalar.dma_start(out=v1t[:], in_=r32(v1[dc * 128 : (dc + 1) * 128, fc * 128 : (fc + 1) * 128]))
                    for n2 in range(2):
                        nc.tensor.matmul(h1w[n2][:], w1t[:], sat[dc][:, HALVES[n2]],
                                         start=(dc == 0), stop=(dc == DC - 1))
                        nc.tensor.matmul(h1v[n2][:], v1t[:], sat[dc][:, HALVES[n2]],
                                         start=(dc == 0), stop=(dc == DC - 1))
                sil = tmp.tile([128, 1024], F32, tag="tmp")
                for n2 in range(2):
                    nc.scalar.activation(sil[:, HALVES[n2]], h1w[n2][:], SILU)
                    nc.vector.tensor_mul(h1c[fc][:, HALVES[n2]], sil[:, HALVES[n2]], h1v[n2][:])

            # ---- P4: outT = yT*rmsy + h1 @ W2   (saT reconstructed) ----
            rmsyB = [ps.tile([128, 512], F32, tag="ps", name=f"rmsyB{n2}") for n2 in range(2)]
            for n2 in range(2):
                nc.tensor.matmul(rmsyB[n2][:], ones1[:], rmsy[:, HALVES[n2]], start=True, stop=True)
            for dg in range(DC // 2):
                h2 = [ps.tile([128, 512], F32, tag="ps", name=f"h2{j}") for j in range(4)]
                for fc in range(FC):
                    w2t = w2st.tile([128, 256], BF16, tag="w2st")
                    nc.scalar.dma_start(out=w2t[:], in_=w2b[fc * 128 : (fc + 1) * 128, dg * 256 : (dg + 1) * 256])
                    for d2 in range(2):
                        for n2 in range(2):
                            nc.tensor.matmul(h2[d2 * 2 + n2][:], w2t[:, d2 * 128 : (d2 + 1) * 128],
                                             h1c[fc][:, HALVES[n2]],
                                             start=(fc == 0), stop=(fc == FC - 1))
                for d2 in range(2):
                    dc2 = dg * 2 + d2
                    ot = tmp.tile([128, 1024], F32, tag="tmp", name="ot")
                    sa_rec = tmp.tile([128, 1024], F32, tag="tmp", name="sa_rec")
                    for n2 in range(2):
                        nc.vector.tensor_mul(sa_rec[:, HALVES[n2]], sat[dc2][:, HALVES[n2]], rmsyB[n2][:])
                        nc.vector.tensor_add(ot[:, HALVES[n2]], sa_rec[:, HALVES[n2]], h2[d2 * 2 + n2][:])
                    nc.sync.dma_start(out=outT[dc2 * 128 : (dc2 + 1) * 128, :], in_=ot[:])

    nc.compile()
    return nc


_NC = None


def _get_nc():
    global _NC
    if _NC is None:
        _NC = build()
    return _NC


def _build_in_maps(inputs):
    return _prep(**inputs)


def kernel(**inputs):
    in_maps = _prep(**inputs)
    res = run_bass_kernel_spmd(_get_nc(), in_maps, list(range(8)))
    out = np.empty((B, S, D), np.float32)
    for c in range(8):
        b, qh = c // 2, c % 2
        out[b, qh * SQ : (qh + 1) * SQ, :] = res.results[c]["outT"].T
    return out


def _prep(src, src_padding_mask, Wq, Wk, Wv, Wo, g1, g2, W1, V1, W2, **_):
    src = np.asarray(src, np.float32)
    valid = (~np.asarray(src_padding_mask, bool)).astype(np.float32)
    g1 = np.asarray(g1, np.float32)
    g2 = np.asarray(g2, np.float32)
    wq_cat = (np.transpose(np.asarray(Wq, np.float32), (1, 0, 2)).reshape(D, D)
              * g1[:, None]).astype(np.float32)
    wk_cat = (np.transpose(np.asarray(Wk, np.float32), (1, 0, 2)).reshape(D, D)
              * g1[:, None]).astype(np.float32)
    wv_cat = (np.transpose(np.asarray(Wv, np.float32), (1, 0, 2)).reshape(D, D)
              * g1[:, None]).astype(np.float32)
    wo_a = np.ascontiguousarray(np.asarray(Wo, np.float32))
    w1_s = np.ascontiguousarray(np.asarray(W1, np.float32) * g2[:, None])
    v1_s = np.ascontiguousarray(np.asarray(V1, np.float32) * g2[:, None])
    w2_b = np.asarray(W2, np.float32).astype(ml_dtypes.bfloat16)

    in_maps = []
    for c in range(8):
        b, qh = c // 2, c % 2
        roll = qh * SQ
        src_r = np.roll(src[b], -roll, axis=0)          # [S, D]
        srcT_c = np.ascontiguousarray(src_r.T)          # [D, S]
        km_c = np.ascontiguousarray(np.roll(valid[b], -roll)).reshape(S, 1)
        # additive softmax-denominator bias: +1e30 on padded query rows so
        # 1/denom ~ 0 there (reference zeroes those attention rows)
        qm_c = np.ascontiguousarray(((1.0 - km_c[0:SQ]) * 1e30).reshape(1, SQ))
        in_maps.append({
            "srcT": srcT_c, "kmask": km_c, "qmask": qm_c,
            "wq": wq_cat, "wk": wk_cat, "wv": wv_cat, "wo": wo_a,
            "w1": w1_s, "v1": v1_s, "w2b": w2_b,
        })
    return in_maps
